# revision 11
# baseline (speedup 1.0000x reference)
"""PiCANet-G attention module as a Trainium2 Bass/Tile kernel.

Pure data-parallel over batch: 64 samples -> 8 cores x 8 samples.

Per core, three phases (all SBUF-resident):
  P1: vertical bi-LSTM over W (batch = 8*28 (b, h) rows, 28 steps, 2 dirs)
  P2: horizontal bi-LSTM over H (batch = 8*28 (b, w) rows)
  P3: fc -> softmax(100) -> per-sample einsum with the dilated 10x10 patch

All LSTM/fc matmuls run in fp8 (e4m3) with DoubleRow perf mode (2 K-tiles
per instruction, 0.5 cycles/row): half the PE time and half the matmul
instruction count vs bf16.  Gates accumulate in fp32 PSUM (one [128,4,512]
tile = 4 banks per direction, gate order i,f,o,g); nonlinearities run on
the Act engine as 3 instructions per (step, dir) (sigmoid over the i+f
banks fused, tanh(g), sigmoid(o)) plus tanh(c).  The element-wise cell
update runs on DVE in fp16 (2x mode).  Hidden state h is written in fp8:
P1 keeps h in a small per-direction ring (contiguous, feeds the next
step's recurrent matmul) while the Pool engine mirrors it into the big
Hv slab in (h, b, w) layout so P2's input matmuls read contiguous 3-D
slices; P2 writes its h directly into the Hh slab (its own recurrent
reads and P3's fc reads are both contiguous there).
"""

import numpy as np
import ml_dtypes
from contextlib import ExitStack

import concourse.bacc as bacc
import concourse.mybir as mybir
import concourse.tile as tile
from concourse.masks import make_identity
from concourse.bass_utils import run_bass_kernel_spmd

# problem shapes (hardcoded per contract)
B, C, H, W = 64, 512, 28, 28
HID = 256
N_CORES = 8
BL = B // N_CORES        # samples per core
NB = BL * H              # 224 rows per LSTM step
T = 28                   # steps per LSTM
PLOC = BL * H * W        # 6272 positions per core

BF16 = mybir.dt.bfloat16
F32 = mybir.dt.float32
F16 = mybir.dt.float16
F8 = mybir.dt.float8e4
AF = mybir.ActivationFunctionType
DR = mybir.MatmulPerfMode.DoubleRow

# torch gate order [i f g o] -> device order [i f o g] (sigmoids first)
_PERM = np.concatenate([np.arange(0, 512), np.arange(768, 1024), np.arange(512, 768)])

_LSTMS = ["vf", "vb", "hf", "hb"]


def _emit_matmuls(nc, pd, wih_sb, whh_sb, src_rhs, hprev, t):
    """PE work for one (step, dir): per gate-half region, a contiguous
    accumulation group of 2 fp8 DoubleRow ih matmuls (+1 hh when t>0)."""
    for g in range(4):
        for h in range(2):
            m = g * 2 + h
            out_ap = pd[:, g, h * 256: h * 256 + 224]
            for q in range(2):
                nc.tensor.matmul(
                    out_ap,
                    lhsT=wih_sb[:, 2 * q:2 * q + 2, m * 128:(m + 1) * 128],
                    rhs=src_rhs(q),
                    start=(q == 0), stop=(t == 0 and q == 1),
                    perf_mode=DR)
            if t > 0:
                nc.tensor.matmul(
                    out_ap,
                    lhsT=whh_sb[:, 0:2, m * 128:(m + 1) * 128],
                    rhs=hprev,
                    start=False, stop=True, perf_mode=DR)


def _emit_act(nc, scr, pd, t, name):
    """Act engine: sigmoid(i,f) fused + tanh(g).  sigmoid(o) is deferred to
    _emit_tail so the Act queue has work between tanh(c) of this step and
    sigmoid(i,f) of the next (which waits on the recurrent matmul)."""
    pdv = pd.rearrange("p g (h x) -> p g h x", h=2)
    IF = scr.tile([128, 2, 2, 224], F16, tag="IF", bufs=3, name=f"IF_{name}")
    nc.scalar.activation(IF, pdv[:, 0:2, :, 0:224], AF.Sigmoid)
    G = scr.tile([128, 2, 224], F16, tag="G", bufs=3, name=f"G_{name}")
    nc.scalar.activation(G, pdv[:, 3, :, 0:224], AF.Tanh)
    return IF, G


def _emit_cell(nc, scr, IF, G, c, t, name):
    """DVE cell update in fp16."""
    if t == 0:
        nc.vector.tensor_mul(c, IF[:, 0], G)
    else:
        nc.vector.tensor_mul(c, IF[:, 1], c)
        t1 = scr.tile([128, 2, 224], F16, tag="t1", bufs=3, name=f"t1_{name}")
        nc.vector.tensor_mul(t1, IF[:, 0], G)
        nc.vector.tensor_add(c, c, t1)


def _emit_tail(nc, scr, pd, c, name):
    """Act: tanh(c) then sigmoid(o); returns (O, th) for the h product."""
    th = scr.tile([128, 2, 224], F16, tag="th", bufs=3, name=f"th_{name}")
    nc.scalar.activation(th, c, AF.Tanh)
    pdv = pd.rearrange("p g (h x) -> p g h x", h=2)
    O = scr.tile([128, 2, 224], F16, tag="O", bufs=3, name=f"O_{name}")
    nc.scalar.activation(O, pdv[:, 2, :, 0:224], AF.Sigmoid)
    return O, th


def _build(reps=1, debug=False, has_bias=False):
    nc = bacc.Bacc(None, target_bir_lowering=False)

    xT_d = nc.dram_tensor("xT", [C, PLOC], F8, kind="ExternalInput")
    w_d = {}
    for L in _LSTMS:
        w_d[L + "_wih"] = nc.dram_tensor(L + "_wih", [512, 1024], F8, kind="ExternalInput")
        w_d[L + "_whh"] = nc.dram_tensor(L + "_whh", [256, 1024], F8, kind="ExternalInput")
        if has_bias:
            w_d[L + "_bias"] = nc.dram_tensor(L + "_bias", [128, 8], F32, kind="ExternalInput")
    fcw_d = nc.dram_tensor("fcw", [512, 100], F8, kind="ExternalInput")
    patchT_d = nc.dram_tensor("patchT", [BL, 100, 512], BF16, kind="ExternalInput")
    out_d = nc.dram_tensor("out", [BL, C, H * W], F32, kind="ExternalOutput")
    if debug:
        dbg_hv = nc.dram_tensor("dbg_hv", [128, 4, PLOC], F8, kind="ExternalOutput")
        dbg_hh = nc.dram_tensor("dbg_hh", [128, 4, PLOC], F8, kind="ExternalOutput")
        dbg_kt = nc.dram_tensor("dbg_kt", [100, PLOC], BF16, kind="ExternalOutput")

    with tile.TileContext(nc) as tc, ExitStack() as ctx:
        wpool = ctx.enter_context(tc.tile_pool(name="wpool", bufs=1))
        bigA = ctx.enter_context(tc.tile_pool(name="bigA", bufs=1))
        bigB = ctx.enter_context(tc.tile_pool(name="bigB", bufs=1))
        state = ctx.enter_context(tc.tile_pool(name="state", bufs=1))
        scr = ctx.enter_context(tc.tile_pool(name="scr", bufs=3))

        # --- load weights; both stage-1 dirs first (step 0 needs them) ---
        wih_sb, whh_sb = {}, {}
        for L in _LSTMS:
            wih_sb[L] = wpool.tile([128, 4, 1024], F8, name=f"wih_{L}")
            whh_sb[L] = wpool.tile([128, 2, 1024], F8, name=f"whh_{L}")
        for L in ["vf", "vb"]:
            nc.sync.dma_start(out=wih_sb[L],
                              in_=w_d[L + "_wih"].rearrange("(kt p) m -> p kt m", kt=4))
        for L in ["vf", "vb"]:
            nc.scalar.dma_start(out=whh_sb[L],
                                in_=w_d[L + "_whh"].rearrange("(kt p) m -> p kt m", kt=2))
        for L in ["hf", "hb"]:
            nc.sync.dma_start(out=wih_sb[L],
                              in_=w_d[L + "_wih"].rearrange("(kt p) m -> p kt m", kt=4))
            nc.sync.dma_start(out=whh_sb[L],
                              in_=w_d[L + "_whh"].rearrange("(kt p) m -> p kt m", kt=2))
        fcw_sb = wpool.tile([128, 4, 100], F8, name="fcw_sb")
        nc.sync.dma_start(out=fcw_sb, in_=fcw_d.rearrange("(kt p) n -> p kt n", kt=4))
        patchT_sb = wpool.tile([100, BL, 512], BF16, name="patchT_sb")
        nc.sync.dma_start(out=patchT_sb, in_=patchT_d.rearrange("b k c -> k b c"))
        ident = wpool.tile([112, 112], F32, name="ident")
        make_identity(nc, ident)

        for rep in range(reps):
            sfx = f"r{rep}"
            # --- P1: vertical bi-LSTM (input cols (w, b, h); out slab (h, b, w)) ---
            xT = bigA.tile([128, 4, PLOC], F8, tag="bigA", name=f"xT_{sfx}")
            xsrc = xT_d.rearrange("(kt p) f -> p kt f", kt=4)
            wblocks = [(0, 3), (25, 28), (3, 8), (20, 25), (8, 14), (14, 20)]
            for lo, hi in wblocks:
                for kk in range(4):
                    nc.scalar.dma_start(out=xT[:, kk, lo * 224:hi * 224],
                                        in_=xsrc[:, kk, lo * 224:hi * 224])
            Hv = bigB.tile([128, 4, PLOC], F8, tag="bigB", name=f"Hv_{sfx}")
            Hv5 = Hv.rearrange("p kt (h b w) -> p kt b h w", h=H, b=BL)

            with tc.tile_pool(name=f"g1{sfx}", bufs=1, space="PSUM") as gpool:
                cs = [state.tile([128, 2, 224], F16, tag=f"c1_{d}",
                                 name=f"c1_{d}_{sfx}") for d in range(2)]
                hprev = [None, None]
                for t in range(T):
                    pds, acts, hcurs = [], [], []
                    for d, L in enumerate(["vf", "vb"]):
                        pos = t if d == 0 else T - 1 - t
                        pd = gpool.tile([128, 4, 512], F32, tag=f"pd{d}",
                                        name=f"pd1_{d}_{t}_{sfx}")
                        _emit_matmuls(nc, pd, wih_sb[L], whh_sb[L],
                                      lambda q, _p=pos: xT[:, 2 * q:2 * q + 2,
                                                           _p * 224:(_p + 1) * 224],
                                      hprev[d], t)
                        pds.append((pd, pos))
                    for d in range(2):
                        acts.append(_emit_act(nc, scr, pds[d][0], t,
                                              f"1{d}_{t}_{sfx}"))
                    for d in range(2):
                        IF, G = acts[d]
                        _emit_cell(nc, scr, IF, G, cs[d], t, f"1{d}_{t}_{sfx}")
                    tails = [_emit_tail(nc, scr, pds[d][0], cs[d],
                                        f"1{d}_{t}_{sfx}") for d in range(2)]
                    for d in range(2):
                        O, th = tails[d]
                        hcur = scr.tile([128, 2, 224], F8, tag=f"ring{d}",
                                        bufs=2, name=f"h1_{d}_{t}_{sfx}")
                        nc.vector.tensor_mul(hcur, O, th)
                        hcurs.append(hcur)
                    for d in range(2):
                        pos = pds[d][1]
                        dst = Hv5[:, 2 * d:2 * d + 2, :, :, pos]
                        src = hcurs[d].rearrange("p kt (b h) -> p kt b h", b=BL)
                        nc.gpsimd.tensor_copy(dst, src)
                        hprev[d] = hcurs[d]

            # --- P2: horizontal bi-LSTM (slab cols (h, b, w) both in and out) ---
            Hh = bigA.tile([128, 4, PLOC], F8, tag="bigA", name=f"Hh_{sfx}")
            with tc.tile_pool(name=f"g2{sfx}", bufs=1, space="PSUM") as gpool:
                cs = [state.tile([128, 2, 224], F16, tag=f"c2_{d}",
                                 name=f"c2_{d}_{sfx}") for d in range(2)]
                hprev = [None, None]
                for t in range(T):
                    pds, acts = [], []
                    for d, L in enumerate(["hf", "hb"]):
                        pos = t if d == 0 else T - 1 - t
                        pd = gpool.tile([128, 4, 512], F32, tag=f"pd{d}",
                                        name=f"pd2_{d}_{t}_{sfx}")
                        _emit_matmuls(nc, pd, wih_sb[L], whh_sb[L],
                                      lambda q, _p=pos: Hv[:, 2 * q:2 * q + 2,
                                                           _p * 224:(_p + 1) * 224],
                                      hprev[d], t)
                        pds.append((pd, pos))
                    for d in range(2):
                        acts.append(_emit_act(nc, scr, pds[d][0], t,
                                              f"2{d}_{t}_{sfx}"))
                    for d in range(2):
                        IF, G = acts[d]
                        _emit_cell(nc, scr, IF, G, cs[d], t, f"2{d}_{t}_{sfx}")
                    tails = [_emit_tail(nc, scr, pds[d][0], cs[d],
                                        f"2{d}_{t}_{sfx}") for d in range(2)]
                    for d in range(2):
                        O, th = tails[d]
                        pos = pds[d][1]
                        hslice = Hh[:, 2 * d:2 * d + 2, pos * 224:(pos + 1) * 224]
                        nc.vector.tensor_mul(hslice, O, th)
                        hprev[d] = hslice

            # --- P3: fc + softmax + transpose + einsum ---
            KT = bigB.tile([100, PLOC], BF16, tag="bigB", name=f"KT_{sfx}")
            with tc.tile_pool(name=f"p3{sfx}", bufs=2, space="PSUM") as pps:
                ci = 0
                for half in range(2):
                    for hr in range(H):
                        off = hr * 224 + half * 112
                        Lp = pps.tile([112, 100], F32, tag="L", name=f"L_{hr}_{half}_{sfx}")
                        for q in range(2):
                            nc.tensor.matmul(Lp,
                                             lhsT=Hh[:, 2 * q:2 * q + 2, off:off + 112],
                                             rhs=fcw_sb[:, 2 * q:2 * q + 2, :],
                                             start=(q == 0), stop=(q == 1),
                                             perf_mode=DR)
                        E = scr.tile([112, 100], F32, tag="E", bufs=3,
                                     name=f"E_{hr}_{half}_{sfx}")
                        Zs = scr.tile([112, 1], F32, tag="Z", bufs=3,
                                      name=f"Z_{hr}_{half}_{sfx}")
                        nc.scalar.activation(E, Lp, AF.Exp, accum_out=Zs)
                        rz = scr.tile([112, 1], F32, tag="rz", bufs=3,
                                      name=f"rz_{hr}_{half}_{sfx}")
                        nc.vector.reciprocal(rz, Zs)
                        Ka = scr.tile([112, 100], F32, tag="Ka", bufs=3,
                                      name=f"Ka_{hr}_{half}_{sfx}")
                        nc.vector.tensor_scalar_mul(Ka, E, rz)
                        KTp = pps.tile([100, 112], F32, tag="KTp",
                                       name=f"KTp_{hr}_{half}_{sfx}")
                        nc.tensor.transpose(KTp, Ka, ident)
                        # KT columns p = b*784 + hr*28 + w for these positions
                        dst = KT.rearrange("k (b hw) -> k b hw", b=BL)[
                            :, half * 4:(half + 1) * 4, hr * 28:(hr + 1) * 28]
                        if ci % 2 == 0:
                            nc.vector.tensor_copy(dst, KTp)
                        else:
                            nc.scalar.copy(dst, KTp)
                        ci += 1
                    # einsum for this half's samples (overlaps the other half's fc)
                    for b_i in range(half * 4, (half + 1) * 4):
                        for ct in range(4):
                            lhsT = patchT_sb[:, b_i, ct * 128:(ct + 1) * 128]
                            Op = pps.tile([128, 2, 512], F32, tag="O", bufs=2,
                                          name=f"O_{b_i}_{ct}_{sfx}")
                            for j2 in range(2):
                                nc.tensor.matmul(
                                    Op[:, j2, 0:392], lhsT=lhsT,
                                    rhs=KT[:, b_i * 784 + j2 * 392:
                                           b_i * 784 + (j2 + 1) * 392],
                                    start=True, stop=True)
                            ob = scr.tile([128, 2, 392], F32, tag="ob", bufs=3,
                                          name=f"ob_{b_i}_{ct}_{sfx}")
                            if ct % 2 == 0:
                                nc.vector.tensor_copy(ob, Op[:, :, 0:392])
                            else:
                                nc.scalar.copy(ob, Op[:, :, 0:392])
                            eng = nc.sync if ct % 2 == 0 else nc.scalar
                            eng.dma_start(
                                out=out_d[b_i, ct * 128:(ct + 1) * 128, :],
                                in_=ob)
            if debug and rep == reps - 1:
                nc.sync.dma_start(out=dbg_hv[:, :, :], in_=Hv)
                nc.sync.dma_start(out=dbg_hh[:, :, :], in_=Hh)
                nc.sync.dma_start(out=dbg_kt[:, :], in_=KT)

    nc.compile()
    return nc


_NC_CACHE = {}


def _get_nc(reps=1, debug=False, has_bias=False):
    key = (reps, debug, has_bias)
    if key not in _NC_CACHE:
        _NC_CACHE[key] = _build(reps=reps, debug=debug, has_bias=has_bias)
    return _NC_CACHE[key]


def _prep_core_inputs(x, weights_np):
    """Host-side marshalling for one core. x: [BL, C, H, W] f32."""
    f8 = ml_dtypes.float8_e4m3
    bf = ml_dtypes.bfloat16
    m = {}
    m["xT"] = np.ascontiguousarray(
        x.transpose(1, 3, 0, 2).reshape(C, PLOC)).astype(f8)
    m["patchT"] = np.ascontiguousarray(
        x[:, :, ::3, ::3].reshape(BL, C, 100).transpose(0, 2, 1)).astype(bf)
    m.update(weights_np)
    return m


def _prep_weights(inputs):
    f8 = ml_dtypes.float8_e4m3
    w = {}
    for L in _LSTMS:
        wih = np.asarray(inputs[L + "_Wih"], np.float32)
        whh = np.asarray(inputs[L + "_Whh"], np.float32)
        w[L + "_wih"] = np.ascontiguousarray(wih[_PERM].T).astype(f8)
        w[L + "_whh"] = np.ascontiguousarray(whh[_PERM].T).astype(f8)
    w["fcw"] = np.asarray(inputs["fc_W"], np.float32).astype(f8)
    return w


def run_cores(inputs, reps=1, debug=False):
    x = np.asarray(inputs["x"], np.float32)
    wnp = _prep_weights(inputs)
    nc = _get_nc(reps=reps, debug=debug)
    in_maps = [
        _prep_core_inputs(x[ci * BL:(ci + 1) * BL], wnp) for ci in range(N_CORES)
    ]
    res = run_bass_kernel_spmd(nc, in_maps, list(range(N_CORES)))
    return res


def kernel(**inputs) -> np.ndarray:
    res = run_cores(inputs)
    out = np.concatenate(
        [res.results[ci]["out"].reshape(BL, C, H, W) for ci in range(N_CORES)],
        axis=0)
    return out.astype(np.float32)


# revision 13
# speedup vs baseline: 1.0805x; 1.0805x over previous
"""PiCANet-G attention module as a Trainium2 Bass/Tile kernel.

Pure data-parallel over batch: 64 samples -> 8 cores x 8 samples.

Per core, three phases (all SBUF-resident):
  P1: vertical bi-LSTM over W (batch = 8*28 (b, h) rows, 28 steps, 2 dirs)
  P2: horizontal bi-LSTM over H (batch = 8*28 (b, w) rows)
  P3: fc -> softmax(100) -> per-sample einsum with the dilated 10x10 patch

All LSTM/fc matmuls run in fp8 (e4m3) with DoubleRow perf mode (2 K-tiles
per instruction, 0.5 cycles/row): half the PE time and half the matmul
instruction count vs bf16.  Gates accumulate in fp32 PSUM (one [128,4,512]
tile = 4 banks per direction, gate order i,f,o,g); nonlinearities run on
the Act engine as 3 instructions per (step, dir) (sigmoid over the i+f
banks fused, tanh(g), sigmoid(o)) plus tanh(c).  The element-wise cell
update runs on DVE in fp16 (2x mode).  Hidden state h is written in fp8:
P1 keeps h in a small per-direction ring (contiguous, feeds the next
step's recurrent matmul) while the Pool engine mirrors it into the big
Hv slab in (h, b, w) layout so P2's input matmuls read contiguous 3-D
slices; P2 writes its h directly into the Hh slab (its own recurrent
reads and P3's fc reads are both contiguous there).
"""

import numpy as np
import ml_dtypes
from contextlib import ExitStack

import concourse.bacc as bacc
import concourse.mybir as mybir
import concourse.tile as tile
from concourse.masks import make_identity
from concourse.bass_utils import run_bass_kernel_spmd

# problem shapes (hardcoded per contract)
B, C, H, W = 64, 512, 28, 28
HID = 256
N_CORES = 8
BL = B // N_CORES        # samples per core
NB = BL * H              # 224 rows per LSTM step
T = 28                   # steps per LSTM
PLOC = BL * H * W        # 6272 positions per core

BF16 = mybir.dt.bfloat16
F32 = mybir.dt.float32
F16 = mybir.dt.float16
F8 = mybir.dt.float8e4
AF = mybir.ActivationFunctionType
DR = mybir.MatmulPerfMode.DoubleRow

# torch gate order [i f g o] -> device order [i f o g] (sigmoids first)
_PERM = np.concatenate([np.arange(0, 512), np.arange(768, 1024), np.arange(512, 768)])

_LSTMS = ["vf", "vb", "hf", "hb"]


def _emit_matmuls(nc, pd, wih_sb, whh_sb, src_rhs, hprev, t):
    """PE work for one (step, dir): per gate-half region, a contiguous
    accumulation group of 2 fp8 DoubleRow ih matmuls (+1 hh when t>0)."""
    for g in range(4):
        for h in range(2):
            m = g * 2 + h
            out_ap = pd[:, g, h * 256: h * 256 + 224]
            for q in range(2):
                nc.tensor.matmul(
                    out_ap,
                    lhsT=wih_sb[:, 2 * q:2 * q + 2, m * 128:(m + 1) * 128],
                    rhs=src_rhs(q),
                    start=(q == 0), stop=(t == 0 and q == 1),
                    perf_mode=DR)
            if t > 0:
                nc.tensor.matmul(
                    out_ap,
                    lhsT=whh_sb[:, 0:2, m * 128:(m + 1) * 128],
                    rhs=hprev,
                    start=False, stop=True, perf_mode=DR)


def _emit_act(nc, scr, pd, t, name):
    """Act engine: sigmoid(i,f) fused, tanh(g), sigmoid(o). Returns tiles."""
    pdv = pd.rearrange("p g (h x) -> p g h x", h=2)
    IF = scr.tile([128, 2, 2, 224], F16, tag="IF", bufs=3, name=f"IF_{name}")
    nc.scalar.activation(IF, pdv[:, 0:2, :, 0:224], AF.Sigmoid)
    G = scr.tile([128, 2, 224], F16, tag="G", bufs=3, name=f"G_{name}")
    nc.scalar.activation(G, pdv[:, 3, :, 0:224], AF.Tanh)
    O = scr.tile([128, 2, 224], F16, tag="O", bufs=3, name=f"O_{name}")
    nc.scalar.activation(O, pdv[:, 2, :, 0:224], AF.Sigmoid)
    return IF, G, O


def _emit_cell(nc, scr, IF, G, c, t, name):
    """DVE cell update in fp16."""
    if t == 0:
        nc.vector.tensor_mul(c, IF[:, 0], G)
    else:
        nc.vector.tensor_mul(c, IF[:, 1], c)
        t1 = scr.tile([128, 2, 224], F16, tag="t1", bufs=3, name=f"t1_{name}")
        nc.vector.tensor_mul(t1, IF[:, 0], G)
        nc.vector.tensor_add(c, c, t1)


def _emit_tail(nc, scr, c, name):
    """Act: tanh(c); returns th for the h product."""
    th = scr.tile([128, 2, 224], F16, tag="th", bufs=3, name=f"th_{name}")
    nc.scalar.activation(th, c, AF.Tanh)
    return th


def _build(reps=1, debug=False, has_bias=False):
    nc = bacc.Bacc(None, target_bir_lowering=False)

    xT_d = nc.dram_tensor("xT", [C, PLOC], F8, kind="ExternalInput")
    w_d = {}
    for L in _LSTMS:
        w_d[L + "_wih"] = nc.dram_tensor(L + "_wih", [512, 1024], F8, kind="ExternalInput")
        w_d[L + "_whh"] = nc.dram_tensor(L + "_whh", [256, 1024], F8, kind="ExternalInput")
        if has_bias:
            w_d[L + "_bias"] = nc.dram_tensor(L + "_bias", [128, 8], F32, kind="ExternalInput")
    fcw_d = nc.dram_tensor("fcw", [512, 100], F8, kind="ExternalInput")
    patchT_d = nc.dram_tensor("patchT", [BL, 100, 512], BF16, kind="ExternalInput")
    out_d = nc.dram_tensor("out", [BL, C, H * W], F32, kind="ExternalOutput")
    if debug:
        dbg_hv = nc.dram_tensor("dbg_hv", [128, 4, PLOC], F8, kind="ExternalOutput")
        dbg_hh = nc.dram_tensor("dbg_hh", [128, 4, PLOC], F8, kind="ExternalOutput")
        dbg_kt = nc.dram_tensor("dbg_kt", [100, PLOC], BF16, kind="ExternalOutput")

    with tile.TileContext(nc) as tc, ExitStack() as ctx:
        wpool = ctx.enter_context(tc.tile_pool(name="wpool", bufs=1))
        bigA = ctx.enter_context(tc.tile_pool(name="bigA", bufs=1))
        bigB = ctx.enter_context(tc.tile_pool(name="bigB", bufs=1))
        state = ctx.enter_context(tc.tile_pool(name="state", bufs=1))
        scr = ctx.enter_context(tc.tile_pool(name="scr", bufs=3))

        # --- load weights; both stage-1 dirs first (step 0 needs them) ---
        wih_sb, whh_sb = {}, {}
        for L in _LSTMS:
            wih_sb[L] = wpool.tile([128, 4, 1024], F8, name=f"wih_{L}")
            whh_sb[L] = wpool.tile([128, 2, 1024], F8, name=f"whh_{L}")
        for L in ["vf", "vb"]:
            nc.sync.dma_start(out=wih_sb[L],
                              in_=w_d[L + "_wih"].rearrange("(kt p) m -> p kt m", kt=4))
        for L in ["vf", "vb"]:
            nc.scalar.dma_start(out=whh_sb[L],
                                in_=w_d[L + "_whh"].rearrange("(kt p) m -> p kt m", kt=2))
        for L in ["hf", "hb"]:
            nc.sync.dma_start(out=wih_sb[L],
                              in_=w_d[L + "_wih"].rearrange("(kt p) m -> p kt m", kt=4))
            nc.sync.dma_start(out=whh_sb[L],
                              in_=w_d[L + "_whh"].rearrange("(kt p) m -> p kt m", kt=2))
        fcw_sb = wpool.tile([128, 4, 100], F8, name="fcw_sb")
        nc.sync.dma_start(out=fcw_sb, in_=fcw_d.rearrange("(kt p) n -> p kt n", kt=4))
        patchT_sb = wpool.tile([100, BL, 512], BF16, name="patchT_sb")
        nc.sync.dma_start(out=patchT_sb, in_=patchT_d.rearrange("b k c -> k b c"))
        ident = wpool.tile([112, 112], F32, name="ident")
        make_identity(nc, ident)

        for rep in range(reps):
            sfx = f"r{rep}"
            # --- P1: vertical bi-LSTM (input cols (w, b, h); out slab (h, b, w)) ---
            xT = bigA.tile([128, 4, PLOC], F8, tag="bigA", name=f"xT_{sfx}")
            xsrc = xT_d.rearrange("(kt p) f -> p kt f", kt=4)
            wblocks = [(0, 3), (25, 28), (3, 8), (20, 25), (8, 14), (14, 20)]
            for lo, hi in wblocks:
                for kk in range(4):
                    nc.scalar.dma_start(out=xT[:, kk, lo * 224:hi * 224],
                                        in_=xsrc[:, kk, lo * 224:hi * 224])
            Hv = bigB.tile([128, 4, PLOC], F8, tag="bigB", name=f"Hv_{sfx}")
            Hv5 = Hv.rearrange("p kt (h b w) -> p kt b h w", h=H, b=BL)

            with tc.tile_pool(name=f"g1{sfx}", bufs=1, space="PSUM") as gpool:
                cs = [state.tile([128, 2, 224], F16, tag=f"c1_{d}",
                                 name=f"c1_{d}_{sfx}") for d in range(2)]
                hprev = [None, None]
                for t in range(T):
                    pds, acts, hcurs = [], [], []
                    for d, L in enumerate(["vf", "vb"]):
                        pos = t if d == 0 else T - 1 - t
                        pd = gpool.tile([128, 4, 512], F32, tag=f"pd{d}",
                                        name=f"pd1_{d}_{t}_{sfx}")
                        _emit_matmuls(nc, pd, wih_sb[L], whh_sb[L],
                                      lambda q, _p=pos: xT[:, 2 * q:2 * q + 2,
                                                           _p * 224:(_p + 1) * 224],
                                      hprev[d], t)
                        pds.append((pd, pos))
                    for d in range(2):
                        acts.append(_emit_act(nc, scr, pds[d][0], t,
                                              f"1{d}_{t}_{sfx}"))
                    for d in range(2):
                        IF, G, O = acts[d]
                        _emit_cell(nc, scr, IF, G, cs[d], t, f"1{d}_{t}_{sfx}")
                    tails = [_emit_tail(nc, scr, cs[d],
                                        f"1{d}_{t}_{sfx}") for d in range(2)]
                    for d in range(2):
                        O = acts[d][2]
                        th = tails[d]
                        hcur = scr.tile([128, 2, 224], F8, tag=f"ring{d}",
                                        bufs=2, name=f"h1_{d}_{t}_{sfx}")
                        nc.vector.tensor_mul(hcur, O, th)
                        hcurs.append(hcur)
                    for d in range(2):
                        pos = pds[d][1]
                        dst = Hv5[:, 2 * d:2 * d + 2, :, :, pos]
                        src = hcurs[d].rearrange("p kt (b h) -> p kt b h", b=BL)
                        nc.gpsimd.tensor_copy(dst, src)
                        hprev[d] = hcurs[d]

            # --- P2: horizontal bi-LSTM (slab cols (h, b, w) both in and out) ---
            Hh = bigA.tile([128, 4, PLOC], F8, tag="bigA", name=f"Hh_{sfx}")
            with tc.tile_pool(name=f"g2{sfx}", bufs=1, space="PSUM") as gpool:
                cs = [state.tile([128, 2, 224], F16, tag=f"c2_{d}",
                                 name=f"c2_{d}_{sfx}") for d in range(2)]
                hprev = [None, None]
                for t in range(T):
                    pds, acts = [], []
                    for d, L in enumerate(["hf", "hb"]):
                        pos = t if d == 0 else T - 1 - t
                        pd = gpool.tile([128, 4, 512], F32, tag=f"pd{d}",
                                        name=f"pd2_{d}_{t}_{sfx}")
                        _emit_matmuls(nc, pd, wih_sb[L], whh_sb[L],
                                      lambda q, _p=pos: Hv[:, 2 * q:2 * q + 2,
                                                           _p * 224:(_p + 1) * 224],
                                      hprev[d], t)
                        pds.append((pd, pos))
                    for d in range(2):
                        acts.append(_emit_act(nc, scr, pds[d][0], t,
                                              f"2{d}_{t}_{sfx}"))
                    for d in range(2):
                        IF, G, O = acts[d]
                        _emit_cell(nc, scr, IF, G, cs[d], t, f"2{d}_{t}_{sfx}")
                    tails = [_emit_tail(nc, scr, cs[d],
                                        f"2{d}_{t}_{sfx}") for d in range(2)]
                    for d in range(2):
                        O = acts[d][2]
                        th = tails[d]
                        pos = pds[d][1]
                        hslice = Hh[:, 2 * d:2 * d + 2, pos * 224:(pos + 1) * 224]
                        nc.vector.tensor_mul(hslice, O, th)
                        hprev[d] = hslice

            # --- P3: fc + softmax + transpose + einsum ---
            KT = bigB.tile([100, PLOC], BF16, tag="bigB", name=f"KT_{sfx}")
            with tc.tile_pool(name=f"p3{sfx}", bufs=2, space="PSUM") as pps:
                ci = 0
                for half in range(2):
                    for hr in range(H):
                        off = hr * 224 + half * 112
                        Lp = pps.tile([112, 100], F32, tag="L", name=f"L_{hr}_{half}_{sfx}")
                        for q in range(2):
                            nc.tensor.matmul(Lp,
                                             lhsT=Hh[:, 2 * q:2 * q + 2, off:off + 112],
                                             rhs=fcw_sb[:, 2 * q:2 * q + 2, :],
                                             start=(q == 0), stop=(q == 1),
                                             perf_mode=DR)
                        E = scr.tile([112, 100], F32, tag="E", bufs=3,
                                     name=f"E_{hr}_{half}_{sfx}")
                        Zs = scr.tile([112, 1], F32, tag="Z", bufs=3,
                                      name=f"Z_{hr}_{half}_{sfx}")
                        nc.scalar.activation(E, Lp, AF.Exp, accum_out=Zs)
                        rz = scr.tile([112, 1], F32, tag="rz", bufs=3,
                                      name=f"rz_{hr}_{half}_{sfx}")
                        nc.vector.reciprocal(rz, Zs)
                        Ka = scr.tile([112, 100], F32, tag="Ka", bufs=3,
                                      name=f"Ka_{hr}_{half}_{sfx}")
                        nc.vector.tensor_scalar_mul(Ka, E, rz)
                        KTp = pps.tile([100, 112], F32, tag="KTp",
                                       name=f"KTp_{hr}_{half}_{sfx}")
                        nc.tensor.transpose(KTp, Ka, ident)
                        # KT columns p = b*784 + hr*28 + w for these positions
                        dst = KT.rearrange("k (b hw) -> k b hw", b=BL)[
                            :, half * 4:(half + 1) * 4, hr * 28:(hr + 1) * 28]
                        if ci % 2 == 0:
                            nc.vector.tensor_copy(dst, KTp)
                        else:
                            nc.scalar.copy(dst, KTp)
                        ci += 1
                    # einsum for this half's samples (overlaps the other half's fc)
                    for b_i in range(half * 4, (half + 1) * 4):
                        for ct in range(4):
                            lhsT = patchT_sb[:, b_i, ct * 128:(ct + 1) * 128]
                            Op = pps.tile([128, 2, 512], F32, tag="O", bufs=2,
                                          name=f"O_{b_i}_{ct}_{sfx}")
                            for j2 in range(2):
                                nc.tensor.matmul(
                                    Op[:, j2, 0:392], lhsT=lhsT,
                                    rhs=KT[:, b_i * 784 + j2 * 392:
                                           b_i * 784 + (j2 + 1) * 392],
                                    start=True, stop=True)
                            ob = scr.tile([128, 2, 392], F32, tag="ob", bufs=3,
                                          name=f"ob_{b_i}_{ct}_{sfx}")
                            if ct % 2 == 0:
                                nc.vector.tensor_copy(ob, Op[:, :, 0:392])
                            else:
                                nc.scalar.copy(ob, Op[:, :, 0:392])
                            eng = nc.sync if ct % 2 == 0 else nc.scalar
                            eng.dma_start(
                                out=out_d[b_i, ct * 128:(ct + 1) * 128, :],
                                in_=ob)
            if debug and rep == reps - 1:
                nc.sync.dma_start(out=dbg_hv[:, :, :], in_=Hv)
                nc.sync.dma_start(out=dbg_hh[:, :, :], in_=Hh)
                nc.sync.dma_start(out=dbg_kt[:, :], in_=KT)

    nc.compile()
    return nc


_NC_CACHE = {}


def _get_nc(reps=1, debug=False, has_bias=False):
    key = (reps, debug, has_bias)
    if key not in _NC_CACHE:
        _NC_CACHE[key] = _build(reps=reps, debug=debug, has_bias=has_bias)
    return _NC_CACHE[key]


def _prep_core_inputs(x, weights_np):
    """Host-side marshalling for one core. x: [BL, C, H, W] f32."""
    f8 = ml_dtypes.float8_e4m3
    bf = ml_dtypes.bfloat16
    m = {}
    m["xT"] = np.ascontiguousarray(
        x.transpose(1, 3, 0, 2).reshape(C, PLOC)).astype(f8)
    m["patchT"] = np.ascontiguousarray(
        x[:, :, ::3, ::3].reshape(BL, C, 100).transpose(0, 2, 1)).astype(bf)
    m.update(weights_np)
    return m


def _prep_weights(inputs):
    f8 = ml_dtypes.float8_e4m3
    w = {}
    for L in _LSTMS:
        wih = np.asarray(inputs[L + "_Wih"], np.float32)
        whh = np.asarray(inputs[L + "_Whh"], np.float32)
        w[L + "_wih"] = np.ascontiguousarray(wih[_PERM].T).astype(f8)
        w[L + "_whh"] = np.ascontiguousarray(whh[_PERM].T).astype(f8)
    w["fcw"] = np.asarray(inputs["fc_W"], np.float32).astype(f8)
    return w


def run_cores(inputs, reps=1, debug=False):
    x = np.asarray(inputs["x"], np.float32)
    wnp = _prep_weights(inputs)
    nc = _get_nc(reps=reps, debug=debug)
    in_maps = [
        _prep_core_inputs(x[ci * BL:(ci + 1) * BL], wnp) for ci in range(N_CORES)
    ]
    res = run_bass_kernel_spmd(nc, in_maps, list(range(N_CORES)))
    return res


def kernel(**inputs) -> np.ndarray:
    res = run_cores(inputs)
    out = np.concatenate(
        [res.results[ci]["out"].reshape(BL, C, H, W) for ci in range(N_CORES)],
        axis=0)
    return out.astype(np.float32)


# revision 15
# speedup vs baseline: 1.0861x; 1.0052x over previous
"""PiCANet-G attention module as a Trainium2 Bass/Tile kernel.

Pure data-parallel over batch: 64 samples -> 8 cores x 8 samples.

Per core, three phases (all SBUF-resident):
  P1: vertical bi-LSTM over W (batch = 8*28 (b, h) rows, 28 steps, 2 dirs)
  P2: horizontal bi-LSTM over H (batch = 8*28 (b, w) rows)
  P3: fc -> softmax(100) -> per-sample einsum with the dilated 10x10 patch

All LSTM/fc matmuls run in fp8 (e4m3) with DoubleRow perf mode (2 K-tiles
per instruction, 0.5 cycles/row): half the PE time and half the matmul
instruction count vs bf16.  Gates accumulate in fp32 PSUM (one [128,4,512]
tile = 4 banks per direction, gate order i,f,o,g); nonlinearities run on
the Act engine as 3 instructions per (step, dir) (sigmoid over the i+f
banks fused, tanh(g), sigmoid(o)) plus tanh(c).  The element-wise cell
update runs on DVE in fp16 (2x mode).  Hidden state h is written in fp8:
P1 keeps h in a small per-direction ring (contiguous, feeds the next
step's recurrent matmul) while the Pool engine mirrors it into the big
Hv slab in (h, b, w) layout so P2's input matmuls read contiguous 3-D
slices; P2 writes its h directly into the Hh slab (its own recurrent
reads and P3's fc reads are both contiguous there).
"""

import numpy as np
import ml_dtypes
from contextlib import ExitStack

import concourse.bacc as bacc
import concourse.mybir as mybir
import concourse.tile as tile
from concourse.masks import make_identity
from concourse.bass_utils import run_bass_kernel_spmd

# problem shapes (hardcoded per contract)
B, C, H, W = 64, 512, 28, 28
HID = 256
N_CORES = 8
BL = B // N_CORES        # samples per core
NB = BL * H              # 224 rows per LSTM step
T = 28                   # steps per LSTM
PLOC = BL * H * W        # 6272 positions per core

BF16 = mybir.dt.bfloat16
F32 = mybir.dt.float32
F16 = mybir.dt.float16
F8 = mybir.dt.float8e4
AF = mybir.ActivationFunctionType
DR = mybir.MatmulPerfMode.DoubleRow

# torch gate order [i f g o] -> device order [i f o g] (sigmoids first)
_PERM = np.concatenate([np.arange(0, 512), np.arange(768, 1024), np.arange(512, 768)])

_LSTMS = ["vf", "vb", "hf", "hb"]


def _emit_matmuls(nc, pd, wih_sb, whh_sb, src_rhs, hprev, t):
    """PE work for one (step, dir): per gate-half region, a contiguous
    accumulation group of 2 fp8 DoubleRow ih matmuls (+1 hh when t>0)."""
    for g in range(4):
        for h in range(2):
            m = g * 2 + h
            out_ap = pd[:, g, h * 256: h * 256 + 224]
            for q in range(2):
                nc.tensor.matmul(
                    out_ap,
                    lhsT=wih_sb[:, 2 * q:2 * q + 2, m * 128:(m + 1) * 128],
                    rhs=src_rhs(q),
                    start=(q == 0), stop=(t == 0 and q == 1),
                    perf_mode=DR)
            if t > 0:
                nc.tensor.matmul(
                    out_ap,
                    lhsT=whh_sb[:, 0:2, m * 128:(m + 1) * 128],
                    rhs=hprev,
                    start=False, stop=True, perf_mode=DR)


def _emit_act(nc, scr, pd, t, name):
    """Act engine: sigmoid(i,f) fused, tanh(g), sigmoid(o). Returns tiles."""
    pdv = pd.rearrange("p g (h x) -> p g h x", h=2)
    IF = scr.tile([128, 2, 2, 224], F16, tag="IF", bufs=3, name=f"IF_{name}")
    nc.scalar.activation(IF, pdv[:, 0:2, :, 0:224], AF.Sigmoid)
    G = scr.tile([128, 2, 224], F16, tag="G", bufs=3, name=f"G_{name}")
    nc.scalar.activation(G, pdv[:, 3, :, 0:224], AF.Tanh)
    O = scr.tile([128, 2, 224], F16, tag="O", bufs=3, name=f"O_{name}")
    nc.scalar.activation(O, pdv[:, 2, :, 0:224], AF.Sigmoid)
    return IF, G, O


def _emit_cell(nc, scr, IF, G, c, t, name):
    """DVE cell update in fp16."""
    if t == 0:
        nc.vector.tensor_mul(c, IF[:, 0], G)
    else:
        nc.vector.tensor_mul(c, IF[:, 1], c)
        t1 = scr.tile([128, 2, 224], F16, tag="t1", bufs=3, name=f"t1_{name}")
        nc.vector.tensor_mul(t1, IF[:, 0], G)
        nc.vector.tensor_add(c, c, t1)


def _emit_tail(nc, scr, c, name):
    """Act: tanh(c); returns th for the h product."""
    th = scr.tile([128, 2, 224], F16, tag="th", bufs=3, name=f"th_{name}")
    nc.scalar.activation(th, c, AF.Tanh)
    return th


def _build(reps=1, debug=False, has_bias=False):
    nc = bacc.Bacc(None, target_bir_lowering=False)

    xT_d = nc.dram_tensor("xT", [C, PLOC], F8, kind="ExternalInput")
    w_d = {}
    for L in _LSTMS:
        w_d[L + "_wih"] = nc.dram_tensor(L + "_wih", [512, 1024], F8, kind="ExternalInput")
        w_d[L + "_whh"] = nc.dram_tensor(L + "_whh", [256, 1024], F8, kind="ExternalInput")
        if has_bias:
            w_d[L + "_bias"] = nc.dram_tensor(L + "_bias", [128, 8], F32, kind="ExternalInput")
    fcw_d = nc.dram_tensor("fcw", [512, 100], F8, kind="ExternalInput")
    patchT_d = nc.dram_tensor("patchT", [BL, 100, 512], BF16, kind="ExternalInput")
    out_d = nc.dram_tensor("out", [BL, C, H * W], F32, kind="ExternalOutput")
    if debug:
        dbg_hv = nc.dram_tensor("dbg_hv", [128, 4, PLOC], F8, kind="ExternalOutput")
        dbg_hh = nc.dram_tensor("dbg_hh", [128, 4, PLOC], F8, kind="ExternalOutput")
        dbg_kt = nc.dram_tensor("dbg_kt", [100, PLOC], BF16, kind="ExternalOutput")

    with tile.TileContext(nc) as tc, ExitStack() as ctx:
        wpool = ctx.enter_context(tc.tile_pool(name="wpool", bufs=1))
        bigA = ctx.enter_context(tc.tile_pool(name="bigA", bufs=1))
        bigB = ctx.enter_context(tc.tile_pool(name="bigB", bufs=1))
        state = ctx.enter_context(tc.tile_pool(name="state", bufs=1))
        scr = ctx.enter_context(tc.tile_pool(name="scr", bufs=3))

        # --- load weights; both stage-1 dirs first (step 0 needs them) ---
        wih_sb, whh_sb = {}, {}
        for L in _LSTMS:
            wih_sb[L] = wpool.tile([128, 4, 1024], F8, name=f"wih_{L}")
            whh_sb[L] = wpool.tile([128, 2, 1024], F8, name=f"whh_{L}")
        # split the P1-critical loads across the sync/gpsimd queues (scalar
        # carries the xT stream); stage-2 weights and fc/patch data trail on
        # the sync queue (only needed at P2/P3)
        vb_src = w_d["vb_wih"].rearrange("(kt p) m -> p kt m", kt=4)
        nc.sync.dma_start(out=wih_sb["vf"],
                          in_=w_d["vf_wih"].rearrange("(kt p) m -> p kt m", kt=4))
        nc.sync.dma_start(out=wih_sb["vb"][:, 0:2], in_=vb_src[:, 0:2])
        for L in ["vf", "vb"]:
            nc.gpsimd.dma_start(out=whh_sb[L],
                                in_=w_d[L + "_whh"].rearrange("(kt p) m -> p kt m", kt=2))
        nc.gpsimd.dma_start(out=wih_sb["vb"][:, 2:4], in_=vb_src[:, 2:4])
        for L in ["hf", "hb"]:
            nc.sync.dma_start(out=wih_sb[L],
                              in_=w_d[L + "_wih"].rearrange("(kt p) m -> p kt m", kt=4))
            nc.sync.dma_start(out=whh_sb[L],
                              in_=w_d[L + "_whh"].rearrange("(kt p) m -> p kt m", kt=2))
        fcw_sb = wpool.tile([128, 4, 100], F8, name="fcw_sb")
        nc.sync.dma_start(out=fcw_sb, in_=fcw_d.rearrange("(kt p) n -> p kt n", kt=4))
        patchT_sb = wpool.tile([100, BL, 512], BF16, name="patchT_sb")
        nc.sync.dma_start(out=patchT_sb, in_=patchT_d.rearrange("b k c -> k b c"))
        ident = wpool.tile([112, 112], F32, name="ident")
        make_identity(nc, ident)
        # warm the Act LUTs during the DMA ramp so the first real activation
        # doesn't pay the table-load latency
        warm = wpool.tile([128, 2], F32, name="warm")
        nc.vector.memset(warm, 0.0)
        nc.scalar.activation(warm[:, 0:1], warm[:, 0:1], AF.Sigmoid)
        nc.scalar.activation(warm[:, 1:2], warm[:, 1:2], AF.Tanh)

        for rep in range(reps):
            sfx = f"r{rep}"
            # --- P1: vertical bi-LSTM (input cols (w, b, h); out slab (h, b, w)) ---
            xT = bigA.tile([128, 4, PLOC], F8, tag="bigA", name=f"xT_{sfx}")
            xsrc = xT_d.rearrange("(kt p) f -> p kt f", kt=4)
            wblocks = [(0, 3), (25, 28), (3, 8), (20, 25), (8, 14), (14, 20)]
            for lo, hi in wblocks:
                for kk in range(4):
                    nc.scalar.dma_start(out=xT[:, kk, lo * 224:hi * 224],
                                        in_=xsrc[:, kk, lo * 224:hi * 224])
            Hv = bigB.tile([128, 4, PLOC], F8, tag="bigB", name=f"Hv_{sfx}")
            Hv5 = Hv.rearrange("p kt (h b w) -> p kt b h w", h=H, b=BL)

            with tc.tile_pool(name=f"g1{sfx}", bufs=1, space="PSUM") as gpool:
                cs = [state.tile([128, 2, 224], F16, tag=f"c1_{d}",
                                 name=f"c1_{d}_{sfx}") for d in range(2)]
                hprev = [None, None]
                for t in range(T):
                    pds, acts, hcurs = [], [], []
                    for d, L in enumerate(["vf", "vb"]):
                        pos = t if d == 0 else T - 1 - t
                        pd = gpool.tile([128, 4, 512], F32, tag=f"pd{d}",
                                        name=f"pd1_{d}_{t}_{sfx}")
                        _emit_matmuls(nc, pd, wih_sb[L], whh_sb[L],
                                      lambda q, _p=pos: xT[:, 2 * q:2 * q + 2,
                                                           _p * 224:(_p + 1) * 224],
                                      hprev[d], t)
                        pds.append((pd, pos))
                    for d in range(2):
                        acts.append(_emit_act(nc, scr, pds[d][0], t,
                                              f"1{d}_{t}_{sfx}"))
                    for d in range(2):
                        IF, G, O = acts[d]
                        _emit_cell(nc, scr, IF, G, cs[d], t, f"1{d}_{t}_{sfx}")
                    tails = [_emit_tail(nc, scr, cs[d],
                                        f"1{d}_{t}_{sfx}") for d in range(2)]
                    for d in range(2):
                        O = acts[d][2]
                        th = tails[d]
                        hcur = scr.tile([128, 2, 224], F8, tag=f"ring{d}",
                                        bufs=2, name=f"h1_{d}_{t}_{sfx}")
                        nc.vector.tensor_mul(hcur, O, th)
                        hcurs.append(hcur)
                    for d in range(2):
                        pos = pds[d][1]
                        dst = Hv5[:, 2 * d:2 * d + 2, :, :, pos]
                        src = hcurs[d].rearrange("p kt (b h) -> p kt b h", b=BL)
                        nc.gpsimd.tensor_copy(dst, src)
                        hprev[d] = hcurs[d]

            # --- P2: horizontal bi-LSTM (slab cols (h, b, w) both in and out) ---
            Hh = bigA.tile([128, 4, PLOC], F8, tag="bigA", name=f"Hh_{sfx}")
            with tc.tile_pool(name=f"g2{sfx}", bufs=1, space="PSUM") as gpool:
                cs = [state.tile([128, 2, 224], F16, tag=f"c2_{d}",
                                 name=f"c2_{d}_{sfx}") for d in range(2)]
                hprev = [None, None]
                for t in range(T):
                    pds, acts = [], []
                    for d, L in enumerate(["hf", "hb"]):
                        pos = t if d == 0 else T - 1 - t
                        pd = gpool.tile([128, 4, 512], F32, tag=f"pd{d}",
                                        name=f"pd2_{d}_{t}_{sfx}")
                        _emit_matmuls(nc, pd, wih_sb[L], whh_sb[L],
                                      lambda q, _p=pos: Hv[:, 2 * q:2 * q + 2,
                                                           _p * 224:(_p + 1) * 224],
                                      hprev[d], t)
                        pds.append((pd, pos))
                    for d in range(2):
                        acts.append(_emit_act(nc, scr, pds[d][0], t,
                                              f"2{d}_{t}_{sfx}"))
                    for d in range(2):
                        IF, G, O = acts[d]
                        _emit_cell(nc, scr, IF, G, cs[d], t, f"2{d}_{t}_{sfx}")
                    tails = [_emit_tail(nc, scr, cs[d],
                                        f"2{d}_{t}_{sfx}") for d in range(2)]
                    for d in range(2):
                        O = acts[d][2]
                        th = tails[d]
                        pos = pds[d][1]
                        hslice = Hh[:, 2 * d:2 * d + 2, pos * 224:(pos + 1) * 224]
                        nc.vector.tensor_mul(hslice, O, th)
                        hprev[d] = hslice

            # --- P3: fc + softmax + transpose + einsum ---
            KT = bigB.tile([100, PLOC], BF16, tag="bigB", name=f"KT_{sfx}")
            with tc.tile_pool(name=f"p3{sfx}", bufs=2, space="PSUM") as pps:
                ci = 0
                for half in range(2):
                    for hr in range(H):
                        off = hr * 224 + half * 112
                        Lp = pps.tile([112, 100], F32, tag="L", name=f"L_{hr}_{half}_{sfx}")
                        for q in range(2):
                            nc.tensor.matmul(Lp,
                                             lhsT=Hh[:, 2 * q:2 * q + 2, off:off + 112],
                                             rhs=fcw_sb[:, 2 * q:2 * q + 2, :],
                                             start=(q == 0), stop=(q == 1),
                                             perf_mode=DR)
                        E = scr.tile([112, 100], F32, tag="E", bufs=3,
                                     name=f"E_{hr}_{half}_{sfx}")
                        Zs = scr.tile([112, 1], F32, tag="Z", bufs=3,
                                      name=f"Z_{hr}_{half}_{sfx}")
                        nc.scalar.activation(E, Lp, AF.Exp, accum_out=Zs)
                        rz = scr.tile([112, 1], F32, tag="rz", bufs=3,
                                      name=f"rz_{hr}_{half}_{sfx}")
                        nc.vector.reciprocal(rz, Zs)
                        Ka = scr.tile([112, 100], F32, tag="Ka", bufs=3,
                                      name=f"Ka_{hr}_{half}_{sfx}")
                        nc.vector.tensor_scalar_mul(Ka, E, rz)
                        KTp = pps.tile([100, 112], F32, tag="KTp",
                                       name=f"KTp_{hr}_{half}_{sfx}")
                        nc.tensor.transpose(KTp, Ka, ident)
                        # KT columns p = b*784 + hr*28 + w for these positions
                        dst = KT.rearrange("k (b hw) -> k b hw", b=BL)[
                            :, half * 4:(half + 1) * 4, hr * 28:(hr + 1) * 28]
                        if ci % 2 == 0:
                            nc.vector.tensor_copy(dst, KTp)
                        else:
                            nc.scalar.copy(dst, KTp)
                        ci += 1
                    # einsum for this half's samples (overlaps the other half's fc)
                    for b_i in range(half * 4, (half + 1) * 4):
                        for ct in range(4):
                            lhsT = patchT_sb[:, b_i, ct * 128:(ct + 1) * 128]
                            Op = pps.tile([128, 2, 512], F32, tag="O", bufs=2,
                                          name=f"O_{b_i}_{ct}_{sfx}")
                            for j2 in range(2):
                                nc.tensor.matmul(
                                    Op[:, j2, 0:392], lhsT=lhsT,
                                    rhs=KT[:, b_i * 784 + j2 * 392:
                                           b_i * 784 + (j2 + 1) * 392],
                                    start=True, stop=True)
                            ob = scr.tile([128, 2, 392], F32, tag="ob", bufs=3,
                                          name=f"ob_{b_i}_{ct}_{sfx}")
                            if ct % 2 == 0:
                                nc.vector.tensor_copy(ob, Op[:, :, 0:392])
                            else:
                                nc.scalar.copy(ob, Op[:, :, 0:392])
                            eng = nc.sync if ct % 2 == 0 else nc.scalar
                            eng.dma_start(
                                out=out_d[b_i, ct * 128:(ct + 1) * 128, :],
                                in_=ob)
            if debug and rep == reps - 1:
                nc.sync.dma_start(out=dbg_hv[:, :, :], in_=Hv)
                nc.sync.dma_start(out=dbg_hh[:, :, :], in_=Hh)
                nc.sync.dma_start(out=dbg_kt[:, :], in_=KT)

    nc.compile()
    return nc


_NC_CACHE = {}


def _get_nc(reps=1, debug=False, has_bias=False):
    key = (reps, debug, has_bias)
    if key not in _NC_CACHE:
        _NC_CACHE[key] = _build(reps=reps, debug=debug, has_bias=has_bias)
    return _NC_CACHE[key]


def _prep_core_inputs(x, weights_np):
    """Host-side marshalling for one core. x: [BL, C, H, W] f32."""
    f8 = ml_dtypes.float8_e4m3
    bf = ml_dtypes.bfloat16
    m = {}
    m["xT"] = np.ascontiguousarray(
        x.transpose(1, 3, 0, 2).reshape(C, PLOC)).astype(f8)
    m["patchT"] = np.ascontiguousarray(
        x[:, :, ::3, ::3].reshape(BL, C, 100).transpose(0, 2, 1)).astype(bf)
    m.update(weights_np)
    return m


def _prep_weights(inputs):
    f8 = ml_dtypes.float8_e4m3
    w = {}
    for L in _LSTMS:
        wih = np.asarray(inputs[L + "_Wih"], np.float32)
        whh = np.asarray(inputs[L + "_Whh"], np.float32)
        w[L + "_wih"] = np.ascontiguousarray(wih[_PERM].T).astype(f8)
        w[L + "_whh"] = np.ascontiguousarray(whh[_PERM].T).astype(f8)
    w["fcw"] = np.asarray(inputs["fc_W"], np.float32).astype(f8)
    return w


def run_cores(inputs, reps=1, debug=False):
    x = np.asarray(inputs["x"], np.float32)
    wnp = _prep_weights(inputs)
    nc = _get_nc(reps=reps, debug=debug)
    in_maps = [
        _prep_core_inputs(x[ci * BL:(ci + 1) * BL], wnp) for ci in range(N_CORES)
    ]
    res = run_bass_kernel_spmd(nc, in_maps, list(range(N_CORES)))
    return res


def kernel(**inputs) -> np.ndarray:
    res = run_cores(inputs)
    out = np.concatenate(
        [res.results[ci]["out"].reshape(BL, C, H, W) for ci in range(N_CORES)],
        axis=0)
    return out.astype(np.float32)


# revision 17
# speedup vs baseline: 1.1136x; 1.0253x over previous
"""PiCANet-G attention module as a Trainium2 Bass/Tile kernel.

Pure data-parallel over batch: 64 samples -> 8 cores x 8 samples.

Per core, three phases (all SBUF-resident):
  P1: vertical bi-LSTM over W (batch = 8*28 (b, h) rows, 28 steps, 2 dirs)
  P2: horizontal bi-LSTM over H (batch = 8*28 (b, w) rows)
  P3: fc -> softmax(100) -> per-sample einsum with the dilated 10x10 patch

All LSTM/fc matmuls run in fp8 (e4m3) with DoubleRow perf mode (2 K-tiles
per instruction, 0.5 cycles/row): half the PE time and half the matmul
instruction count vs bf16.  Gates accumulate in fp32 PSUM (one [128,4,512]
tile = 4 banks per direction, gate order i,f,o,g); nonlinearities run on
the Act engine as 3 instructions per (step, dir) (sigmoid over the i+f
banks fused, tanh(g), sigmoid(o)) plus tanh(c).  The element-wise cell
update runs on DVE in fp16 (2x mode).  Hidden state h is written in fp8:
P1 keeps h in a small per-direction ring (contiguous, feeds the next
step's recurrent matmul) while the Pool engine mirrors it into the big
Hv slab in (h, b, w) layout so P2's input matmuls read contiguous 3-D
slices; P2 writes its h directly into the Hh slab (its own recurrent
reads and P3's fc reads are both contiguous there).
"""

import numpy as np
import ml_dtypes
from contextlib import ExitStack

import concourse.bacc as bacc
import concourse.mybir as mybir
import concourse.tile as tile
from concourse.masks import make_identity
from concourse.bass_utils import run_bass_kernel_spmd

# problem shapes (hardcoded per contract)
B, C, H, W = 64, 512, 28, 28
HID = 256
N_CORES = 8
BL = B // N_CORES        # samples per core
NB = BL * H              # 224 rows per LSTM step
T = 28                   # steps per LSTM
PLOC = BL * H * W        # 6272 positions per core

BF16 = mybir.dt.bfloat16
F32 = mybir.dt.float32
F16 = mybir.dt.float16
F8 = mybir.dt.float8e4
AF = mybir.ActivationFunctionType
DR = mybir.MatmulPerfMode.DoubleRow

# torch gate order [i f g o] -> device order [i f o g] (sigmoids first)
_PERM = np.concatenate([np.arange(0, 512), np.arange(768, 1024), np.arange(512, 768)])

_LSTMS = ["vf", "vb", "hf", "hb"]


def _emit_matmuls(nc, pd, wih_sb, whh_sb, src_rhs, hprev, t):
    """PE work for one (step, dir): per gate-half region, a contiguous
    accumulation group of 2 fp8 DoubleRow ih matmuls (+1 hh when t>0)."""
    for g in range(4):
        for h in range(2):
            m = g * 2 + h
            out_ap = pd[:, g, h * 256: h * 256 + 224]
            for q in range(2):
                nc.tensor.matmul(
                    out_ap,
                    lhsT=wih_sb[:, 2 * q:2 * q + 2, m * 128:(m + 1) * 128],
                    rhs=src_rhs(q),
                    start=(q == 0), stop=(t == 0 and q == 1),
                    perf_mode=DR)
            if t > 0:
                nc.tensor.matmul(
                    out_ap,
                    lhsT=whh_sb[:, 0:2, m * 128:(m + 1) * 128],
                    rhs=hprev,
                    start=False, stop=True, perf_mode=DR)


def _emit_act(nc, scr, pd, t, name):
    """Act engine: sigmoid(i,f) fused, tanh(g), sigmoid(o). Returns tiles."""
    pdv = pd.rearrange("p g (h x) -> p g h x", h=2)
    IF = scr.tile([128, 2, 2, 224], F16, tag="IF", bufs=3, name=f"IF_{name}")
    nc.scalar.activation(IF, pdv[:, 0:2, :, 0:224], AF.Sigmoid)
    G = scr.tile([128, 2, 224], F16, tag="G", bufs=3, name=f"G_{name}")
    nc.scalar.activation(G, pdv[:, 3, :, 0:224], AF.Tanh)
    O = scr.tile([128, 2, 224], F16, tag="O", bufs=3, name=f"O_{name}")
    nc.scalar.activation(O, pdv[:, 2, :, 0:224], AF.Sigmoid)
    return IF, G, O


def _emit_cell(nc, scr, IF, G, c, t, name):
    """DVE cell update in fp16."""
    if t == 0:
        nc.vector.tensor_mul(c, IF[:, 0], G)
    else:
        nc.vector.tensor_mul(c, IF[:, 1], c)
        t1 = scr.tile([128, 2, 224], F16, tag="t1", bufs=3, name=f"t1_{name}")
        nc.vector.tensor_mul(t1, IF[:, 0], G)
        nc.vector.tensor_add(c, c, t1)


def _emit_tail(nc, scr, c, name):
    """Act: tanh(c); returns th for the h product."""
    th = scr.tile([128, 2, 224], F16, tag="th", bufs=3, name=f"th_{name}")
    nc.scalar.activation(th, c, AF.Tanh)
    return th


def _build(reps=1, debug=False, has_bias=False):
    nc = bacc.Bacc(None, target_bir_lowering=False)

    xT_d = nc.dram_tensor("xT", [C, PLOC], F8, kind="ExternalInput")
    w_d = {}
    for L in _LSTMS:
        w_d[L + "_wih"] = nc.dram_tensor(L + "_wih", [512, 1024], F8, kind="ExternalInput")
        w_d[L + "_whh"] = nc.dram_tensor(L + "_whh", [256, 1024], F8, kind="ExternalInput")
        if has_bias:
            w_d[L + "_bias"] = nc.dram_tensor(L + "_bias", [128, 8], F32, kind="ExternalInput")
    fcw_d = nc.dram_tensor("fcw", [512, 100], F8, kind="ExternalInput")
    patchT_d = nc.dram_tensor("patchT", [BL, 100, 512], BF16, kind="ExternalInput")
    out_d = nc.dram_tensor("out", [BL, C, H * W], F32, kind="ExternalOutput")
    if debug:
        dbg_hv = nc.dram_tensor("dbg_hv", [128, 4, PLOC], F8, kind="ExternalOutput")
        dbg_hh = nc.dram_tensor("dbg_hh", [128, 4, PLOC], F8, kind="ExternalOutput")
        dbg_kt = nc.dram_tensor("dbg_kt", [100, PLOC], BF16, kind="ExternalOutput")

    with tile.TileContext(nc) as tc, ExitStack() as ctx:
        wpool = ctx.enter_context(tc.tile_pool(name="wpool", bufs=1))
        bigA = ctx.enter_context(tc.tile_pool(name="bigA", bufs=1))
        bigB = ctx.enter_context(tc.tile_pool(name="bigB", bufs=1))
        state = ctx.enter_context(tc.tile_pool(name="state", bufs=1))
        scr = ctx.enter_context(tc.tile_pool(name="scr", bufs=3))

        # --- load weights; both stage-1 dirs first (step 0 needs them) ---
        wih_sb, whh_sb = {}, {}
        for L in _LSTMS:
            wih_sb[L] = wpool.tile([128, 4, 1024], F8, name=f"wih_{L}")
            whh_sb[L] = wpool.tile([128, 2, 1024], F8, name=f"whh_{L}")
        # split the P1-critical loads across the sync/gpsimd queues (scalar
        # carries the xT stream); stage-2 weights and fc/patch data trail on
        # the sync queue (only needed at P2/P3)
        vb_src = w_d["vb_wih"].rearrange("(kt p) m -> p kt m", kt=4)
        nc.sync.dma_start(out=wih_sb["vf"],
                          in_=w_d["vf_wih"].rearrange("(kt p) m -> p kt m", kt=4))
        nc.sync.dma_start(out=wih_sb["vb"][:, 0:2], in_=vb_src[:, 0:2])
        for L in ["vf", "vb"]:
            nc.gpsimd.dma_start(out=whh_sb[L],
                                in_=w_d[L + "_whh"].rearrange("(kt p) m -> p kt m", kt=2))
        nc.gpsimd.dma_start(out=wih_sb["vb"][:, 2:4], in_=vb_src[:, 2:4])
        for L in ["hf", "hb"]:
            nc.sync.dma_start(out=wih_sb[L],
                              in_=w_d[L + "_wih"].rearrange("(kt p) m -> p kt m", kt=4))
            nc.sync.dma_start(out=whh_sb[L],
                              in_=w_d[L + "_whh"].rearrange("(kt p) m -> p kt m", kt=2))
        fcw_sb = wpool.tile([128, 4, 100], F8, name="fcw_sb")
        nc.sync.dma_start(out=fcw_sb, in_=fcw_d.rearrange("(kt p) n -> p kt n", kt=4))
        patchT_sb = wpool.tile([100, BL, 512], BF16, name="patchT_sb")
        nc.sync.dma_start(out=patchT_sb, in_=patchT_d.rearrange("b k c -> k b c"))
        ident = wpool.tile([112, 112], F32, name="ident")
        make_identity(nc, ident)
        # warm the Act LUTs during the DMA ramp so the first real activation
        # doesn't pay the table-load latency
        warm = wpool.tile([128, 2], F32, name="warm")
        nc.vector.memset(warm, 0.0)
        nc.scalar.activation(warm[:, 0:1], warm[:, 0:1], AF.Sigmoid)
        nc.scalar.activation(warm[:, 1:2], warm[:, 1:2], AF.Tanh)

        for rep in range(reps):
            sfx = f"r{rep}"
            # --- P1: vertical bi-LSTM (input cols (w, b, h); out slab (h, b, w)) ---
            xT = bigA.tile([128, 4, PLOC], F8, tag="bigA", name=f"xT_{sfx}")
            xsrc = xT_d.rearrange("(kt p) f -> p kt f", kt=4)
            # scalar queue: the early-critical blocks in consumption order;
            # gpsimd queue (behind the P1 weights): the late middle blocks
            for eng, blocks in [(nc.scalar, [(0, 2), (26, 28), (2, 8), (8, 14)]),
                                (nc.gpsimd, [(20, 26), (14, 20)])]:
                for lo, hi in blocks:
                    eng.dma_start(out=xT[:, :, lo * 224:hi * 224],
                                  in_=xsrc[:, :, lo * 224:hi * 224])
            Hv = bigB.tile([128, 4, PLOC], F8, tag="bigB", name=f"Hv_{sfx}")
            Hv5 = Hv.rearrange("p kt (h b w) -> p kt b h w", h=H, b=BL)

            with tc.tile_pool(name=f"g1{sfx}", bufs=1, space="PSUM") as gpool:
                cs = [state.tile([128, 2, 224], F16, tag=f"c1_{d}",
                                 name=f"c1_{d}_{sfx}") for d in range(2)]
                hprev = [None, None]
                for t in range(T):
                    pds, acts, hcurs = [], [], []
                    for d, L in enumerate(["vf", "vb"]):
                        pos = t if d == 0 else T - 1 - t
                        pd = gpool.tile([128, 4, 512], F32, tag=f"pd{d}",
                                        name=f"pd1_{d}_{t}_{sfx}")
                        _emit_matmuls(nc, pd, wih_sb[L], whh_sb[L],
                                      lambda q, _p=pos: xT[:, 2 * q:2 * q + 2,
                                                           _p * 224:(_p + 1) * 224],
                                      hprev[d], t)
                        pds.append((pd, pos))
                    for d in range(2):
                        acts.append(_emit_act(nc, scr, pds[d][0], t,
                                              f"1{d}_{t}_{sfx}"))
                    for d in range(2):
                        IF, G, O = acts[d]
                        _emit_cell(nc, scr, IF, G, cs[d], t, f"1{d}_{t}_{sfx}")
                    tails = [_emit_tail(nc, scr, cs[d],
                                        f"1{d}_{t}_{sfx}") for d in range(2)]
                    for d in range(2):
                        O = acts[d][2]
                        th = tails[d]
                        hcur = scr.tile([128, 2, 224], F8, tag=f"ring{d}",
                                        bufs=2, name=f"h1_{d}_{t}_{sfx}")
                        nc.vector.tensor_mul(hcur, O, th)
                        hcurs.append(hcur)
                    for d in range(2):
                        pos = pds[d][1]
                        dst = Hv5[:, 2 * d:2 * d + 2, :, :, pos]
                        src = hcurs[d].rearrange("p kt (b h) -> p kt b h", b=BL)
                        nc.gpsimd.tensor_copy(dst, src)
                        hprev[d] = hcurs[d]

            # --- P2: horizontal bi-LSTM (slab cols (h, b, w) both in and out) ---
            Hh = bigA.tile([128, 4, PLOC], F8, tag="bigA", name=f"Hh_{sfx}")
            with tc.tile_pool(name=f"g2{sfx}", bufs=1, space="PSUM") as gpool:
                cs = [state.tile([128, 2, 224], F16, tag=f"c2_{d}",
                                 name=f"c2_{d}_{sfx}") for d in range(2)]
                hprev = [None, None]
                for t in range(T):
                    pds, acts = [], []
                    for d, L in enumerate(["hf", "hb"]):
                        pos = t if d == 0 else T - 1 - t
                        pd = gpool.tile([128, 4, 512], F32, tag=f"pd{d}",
                                        name=f"pd2_{d}_{t}_{sfx}")
                        _emit_matmuls(nc, pd, wih_sb[L], whh_sb[L],
                                      lambda q, _p=pos: Hv[:, 2 * q:2 * q + 2,
                                                           _p * 224:(_p + 1) * 224],
                                      hprev[d], t)
                        pds.append((pd, pos))
                    for d in range(2):
                        acts.append(_emit_act(nc, scr, pds[d][0], t,
                                              f"2{d}_{t}_{sfx}"))
                    for d in range(2):
                        IF, G, O = acts[d]
                        _emit_cell(nc, scr, IF, G, cs[d], t, f"2{d}_{t}_{sfx}")
                    tails = [_emit_tail(nc, scr, cs[d],
                                        f"2{d}_{t}_{sfx}") for d in range(2)]
                    for d in range(2):
                        O = acts[d][2]
                        th = tails[d]
                        pos = pds[d][1]
                        hslice = Hh[:, 2 * d:2 * d + 2, pos * 224:(pos + 1) * 224]
                        nc.vector.tensor_mul(hslice, O, th)
                        hprev[d] = hslice

            # --- P3: fc + softmax + transpose + einsum ---
            KT = bigB.tile([100, PLOC], BF16, tag="bigB", name=f"KT_{sfx}")
            with tc.tile_pool(name=f"p3{sfx}", bufs=2, space="PSUM") as pps:
                ci = 0
                for half in range(2):
                    for hr in range(H):
                        off = hr * 224 + half * 112
                        Lp = pps.tile([112, 100], F32, tag="L", name=f"L_{hr}_{half}_{sfx}")
                        for q in range(2):
                            nc.tensor.matmul(Lp,
                                             lhsT=Hh[:, 2 * q:2 * q + 2, off:off + 112],
                                             rhs=fcw_sb[:, 2 * q:2 * q + 2, :],
                                             start=(q == 0), stop=(q == 1),
                                             perf_mode=DR)
                        E = scr.tile([112, 100], F32, tag="E", bufs=3,
                                     name=f"E_{hr}_{half}_{sfx}")
                        Zs = scr.tile([112, 1], F32, tag="Z", bufs=3,
                                      name=f"Z_{hr}_{half}_{sfx}")
                        nc.scalar.activation(E, Lp, AF.Exp, accum_out=Zs)
                        rz = scr.tile([112, 1], F32, tag="rz", bufs=3,
                                      name=f"rz_{hr}_{half}_{sfx}")
                        nc.vector.reciprocal(rz, Zs)
                        Ka = scr.tile([112, 100], F32, tag="Ka", bufs=3,
                                      name=f"Ka_{hr}_{half}_{sfx}")
                        nc.vector.tensor_scalar_mul(Ka, E, rz)
                        KTp = pps.tile([100, 112], F32, tag="KTp",
                                       name=f"KTp_{hr}_{half}_{sfx}")
                        nc.tensor.transpose(KTp, Ka, ident)
                        # KT columns p = b*784 + hr*28 + w for these positions
                        dst = KT.rearrange("k (b hw) -> k b hw", b=BL)[
                            :, half * 4:(half + 1) * 4, hr * 28:(hr + 1) * 28]
                        if ci % 2 == 0:
                            nc.vector.tensor_copy(dst, KTp)
                        else:
                            nc.scalar.copy(dst, KTp)
                        ci += 1
                    # einsum for this half's samples (overlaps the other half's fc)
                    for b_i in range(half * 4, (half + 1) * 4):
                        for ct in range(4):
                            lhsT = patchT_sb[:, b_i, ct * 128:(ct + 1) * 128]
                            Op = pps.tile([128, 2, 512], F32, tag="O", bufs=2,
                                          name=f"O_{b_i}_{ct}_{sfx}")
                            for j2 in range(2):
                                nc.tensor.matmul(
                                    Op[:, j2, 0:392], lhsT=lhsT,
                                    rhs=KT[:, b_i * 784 + j2 * 392:
                                           b_i * 784 + (j2 + 1) * 392],
                                    start=True, stop=True)
                            ob = scr.tile([128, 2, 392], F32, tag="ob", bufs=3,
                                          name=f"ob_{b_i}_{ct}_{sfx}")
                            if ct % 2 == 0:
                                nc.vector.tensor_copy(ob, Op[:, :, 0:392])
                            else:
                                nc.scalar.copy(ob, Op[:, :, 0:392])
                            eng = nc.sync if ct % 2 == 0 else nc.scalar
                            eng.dma_start(
                                out=out_d[b_i, ct * 128:(ct + 1) * 128, :],
                                in_=ob)
            if debug and rep == reps - 1:
                nc.sync.dma_start(out=dbg_hv[:, :, :], in_=Hv)
                nc.sync.dma_start(out=dbg_hh[:, :, :], in_=Hh)
                nc.sync.dma_start(out=dbg_kt[:, :], in_=KT)

    nc.compile()
    return nc


_NC_CACHE = {}


def _get_nc(reps=1, debug=False, has_bias=False):
    key = (reps, debug, has_bias)
    if key not in _NC_CACHE:
        _NC_CACHE[key] = _build(reps=reps, debug=debug, has_bias=has_bias)
    return _NC_CACHE[key]


def _prep_core_inputs(x, weights_np):
    """Host-side marshalling for one core. x: [BL, C, H, W] f32."""
    f8 = ml_dtypes.float8_e4m3
    bf = ml_dtypes.bfloat16
    m = {}
    m["xT"] = np.ascontiguousarray(
        x.transpose(1, 3, 0, 2).reshape(C, PLOC)).astype(f8)
    m["patchT"] = np.ascontiguousarray(
        x[:, :, ::3, ::3].reshape(BL, C, 100).transpose(0, 2, 1)).astype(bf)
    m.update(weights_np)
    return m


def _prep_weights(inputs):
    f8 = ml_dtypes.float8_e4m3
    w = {}
    for L in _LSTMS:
        wih = np.asarray(inputs[L + "_Wih"], np.float32)
        whh = np.asarray(inputs[L + "_Whh"], np.float32)
        w[L + "_wih"] = np.ascontiguousarray(wih[_PERM].T).astype(f8)
        w[L + "_whh"] = np.ascontiguousarray(whh[_PERM].T).astype(f8)
    w["fcw"] = np.asarray(inputs["fc_W"], np.float32).astype(f8)
    return w


def run_cores(inputs, reps=1, debug=False):
    x = np.asarray(inputs["x"], np.float32)
    wnp = _prep_weights(inputs)
    nc = _get_nc(reps=reps, debug=debug)
    in_maps = [
        _prep_core_inputs(x[ci * BL:(ci + 1) * BL], wnp) for ci in range(N_CORES)
    ]
    res = run_bass_kernel_spmd(nc, in_maps, list(range(N_CORES)))
    return res


def kernel(**inputs) -> np.ndarray:
    res = run_cores(inputs)
    out = np.concatenate(
        [res.results[ci]["out"].reshape(BL, C, H, W) for ci in range(N_CORES)],
        axis=0)
    return out.astype(np.float32)


# revision 19
# speedup vs baseline: 1.1643x; 1.0455x over previous
"""PiCANet-G attention module as a Trainium2 Bass/Tile kernel.

Pure data-parallel over batch: 64 samples -> 8 cores x 8 samples.

Per core, three phases (all SBUF-resident):
  P1: vertical bi-LSTM over W (batch = 8*28 (b, h) rows, 28 steps, 2 dirs)
  P2: horizontal bi-LSTM over H (batch = 8*28 (b, w) rows)
  P3: fc -> softmax(100) -> per-sample einsum with the dilated 10x10 patch

All LSTM/fc matmuls run in fp8 (e4m3) with DoubleRow perf mode (2 K-tiles
per instruction, 0.5 cycles/row): half the PE time and half the matmul
instruction count vs bf16.  Gates accumulate in fp32 PSUM (one [128,4,512]
tile = 4 banks per direction, gate order i,f,o,g); nonlinearities run on
the Act engine as 3 instructions per (step, dir) (sigmoid over the i+f
banks fused, tanh(g), sigmoid(o)) plus tanh(c).  The element-wise cell
update runs on DVE in fp16 (2x mode).  Hidden state h is written in fp8:
P1 keeps h in a small per-direction ring (contiguous, feeds the next
step's recurrent matmul) while the Pool engine mirrors it into the big
Hv slab in (h, b, w) layout so P2's input matmuls read contiguous 3-D
slices; P2 writes its h directly into the Hh slab (its own recurrent
reads and P3's fc reads are both contiguous there).
"""

import numpy as np
import ml_dtypes
from contextlib import ExitStack

import concourse.bacc as bacc
import concourse.mybir as mybir
import concourse.tile as tile
from concourse.masks import make_identity
from concourse.bass_utils import run_bass_kernel_spmd

# problem shapes (hardcoded per contract)
B, C, H, W = 64, 512, 28, 28
HID = 256
N_CORES = 8
BL = B // N_CORES        # samples per core
NB = BL * H              # 224 rows per LSTM step
T = 28                   # steps per LSTM
PLOC = BL * H * W        # 6272 positions per core

BF16 = mybir.dt.bfloat16
F32 = mybir.dt.float32
F16 = mybir.dt.float16
F8 = mybir.dt.float8e4
AF = mybir.ActivationFunctionType
DR = mybir.MatmulPerfMode.DoubleRow

# torch gate order [i f g o] -> device order [i f o g] (sigmoids first)
_PERM = np.concatenate([np.arange(0, 512), np.arange(768, 1024), np.arange(512, 768)])

_LSTMS = ["vf", "vb", "hf", "hb"]


def _emit_matmuls(nc, pd, wih_sb, whh_sb, src_rhs, hprev, t):
    """PE work for one (step, dir): per gate-half region, a contiguous
    accumulation group of 2 fp8 DoubleRow ih matmuls (+1 hh when t>0)."""
    for g in range(4):
        for h in range(2):
            m = g * 2 + h
            out_ap = pd[:, g, h * 256: h * 256 + 224]
            for q in range(2):
                nc.tensor.matmul(
                    out_ap,
                    lhsT=wih_sb[:, 2 * q:2 * q + 2, m * 128:(m + 1) * 128],
                    rhs=src_rhs(q),
                    start=(q == 0), stop=(t == 0 and q == 1),
                    perf_mode=DR)
            if t > 0:
                nc.tensor.matmul(
                    out_ap,
                    lhsT=whh_sb[:, 0:2, m * 128:(m + 1) * 128],
                    rhs=hprev,
                    start=False, stop=True, perf_mode=DR)


def _emit_act(nc, scr, pd, t, name):
    """Act engine: sigmoid(i,f) fused, tanh(g), sigmoid(o). Returns tiles."""
    pdv = pd.rearrange("p g (h x) -> p g h x", h=2)
    IF = scr.tile([128, 2, 2, 224], F16, tag="IF", bufs=3, name=f"IF_{name}")
    nc.scalar.activation(IF, pdv[:, 0:2, :, 0:224], AF.Sigmoid)
    G = scr.tile([128, 2, 224], F16, tag="G", bufs=3, name=f"G_{name}")
    nc.scalar.activation(G, pdv[:, 3, :, 0:224], AF.Tanh)
    O = scr.tile([128, 2, 224], F16, tag="O", bufs=3, name=f"O_{name}")
    nc.scalar.activation(O, pdv[:, 2, :, 0:224], AF.Sigmoid)
    return IF, G, O


def _emit_cell(nc, scr, IF, G, c, t, name):
    """DVE cell update in fp16."""
    if t == 0:
        nc.vector.tensor_mul(c, IF[:, 0], G)
    else:
        nc.vector.tensor_mul(c, IF[:, 1], c)
        t1 = scr.tile([128, 2, 224], F16, tag="t1", bufs=3, name=f"t1_{name}")
        nc.vector.tensor_mul(t1, IF[:, 0], G)
        nc.vector.tensor_add(c, c, t1)


def _emit_tail(nc, scr, c, name):
    """Act: tanh(c); returns th for the h product."""
    th = scr.tile([128, 2, 224], F16, tag="th", bufs=3, name=f"th_{name}")
    nc.scalar.activation(th, c, AF.Tanh)
    return th


def _build(reps=1, debug=False, has_bias=False):
    nc = bacc.Bacc(None, target_bir_lowering=False)

    xT_d = nc.dram_tensor("xT", [C, PLOC], F8, kind="ExternalInput")
    w_d = {}
    for L in _LSTMS:
        w_d[L + "_wih"] = nc.dram_tensor(L + "_wih", [512, 1024], F8, kind="ExternalInput")
        w_d[L + "_whh"] = nc.dram_tensor(L + "_whh", [256, 1024], F8, kind="ExternalInput")
        if has_bias:
            w_d[L + "_bias"] = nc.dram_tensor(L + "_bias", [128, 8], F32, kind="ExternalInput")
    fcw_d = nc.dram_tensor("fcw", [512, 100], F8, kind="ExternalInput")
    patchT_d = nc.dram_tensor("patchT", [BL, 100, 512], BF16, kind="ExternalInput")
    out_d = nc.dram_tensor("out", [BL, C, H * W], F32, kind="ExternalOutput")
    if debug:
        dbg_hv = nc.dram_tensor("dbg_hv", [128, 4, PLOC], F8, kind="ExternalOutput")
        dbg_hh = nc.dram_tensor("dbg_hh", [128, 4, PLOC], F8, kind="ExternalOutput")
        dbg_kt = nc.dram_tensor("dbg_kt", [100, PLOC], BF16, kind="ExternalOutput")

    with tile.TileContext(nc) as tc, ExitStack() as ctx:
        wpool = ctx.enter_context(tc.tile_pool(name="wpool", bufs=1))
        bigA = ctx.enter_context(tc.tile_pool(name="bigA", bufs=1))
        bigB = ctx.enter_context(tc.tile_pool(name="bigB", bufs=1))
        state = ctx.enter_context(tc.tile_pool(name="state", bufs=1))
        scr = ctx.enter_context(tc.tile_pool(name="scr", bufs=3))

        # --- load weights; both stage-1 dirs first (step 0 needs them) ---
        wih_sb, whh_sb = {}, {}
        for L in _LSTMS:
            wih_sb[L] = wpool.tile([128, 4, 1024], F8, name=f"wih_{L}")
            whh_sb[L] = wpool.tile([128, 2, 1024], F8, name=f"whh_{L}")
        # split the P1-critical loads across the sync/gpsimd queues (scalar
        # carries the xT stream); stage-2 weights and fc/patch data trail on
        # the sync queue (only needed at P2/P3)
        vb_src = w_d["vb_wih"].rearrange("(kt p) m -> p kt m", kt=4)
        nc.sync.dma_start(out=wih_sb["vf"],
                          in_=w_d["vf_wih"].rearrange("(kt p) m -> p kt m", kt=4))
        nc.sync.dma_start(out=wih_sb["vb"][:, 0:2], in_=vb_src[:, 0:2])
        for L in ["vf", "vb"]:
            nc.gpsimd.dma_start(out=whh_sb[L],
                                in_=w_d[L + "_whh"].rearrange("(kt p) m -> p kt m", kt=2))
        nc.gpsimd.dma_start(out=wih_sb["vb"][:, 2:4], in_=vb_src[:, 2:4])
        for L in ["hf", "hb"]:
            nc.sync.dma_start(out=wih_sb[L],
                              in_=w_d[L + "_wih"].rearrange("(kt p) m -> p kt m", kt=4))
            nc.sync.dma_start(out=whh_sb[L],
                              in_=w_d[L + "_whh"].rearrange("(kt p) m -> p kt m", kt=2))
        fcw_sb = wpool.tile([128, 4, 100], F8, name="fcw_sb")
        nc.sync.dma_start(out=fcw_sb, in_=fcw_d.rearrange("(kt p) n -> p kt n", kt=4))
        patchT_sb = wpool.tile([100, BL, 512], BF16, name="patchT_sb")
        nc.sync.dma_start(out=patchT_sb, in_=patchT_d.rearrange("b k c -> k b c"))
        ident = wpool.tile([112, 112], F32, name="ident")
        make_identity(nc, ident)
        # warm the Act LUTs during the DMA ramp so the first real activation
        # doesn't pay the table-load latency
        warm = wpool.tile([128, 2], F32, name="warm")
        nc.vector.memset(warm, 0.0)
        nc.scalar.activation(warm[:, 0:1], warm[:, 0:1], AF.Sigmoid)
        nc.scalar.activation(warm[:, 1:2], warm[:, 1:2], AF.Tanh)

        for rep in range(reps):
            sfx = f"r{rep}"
            # --- P1: vertical bi-LSTM (input cols (w, b, h); out slab (h, b, w)) ---
            xT = bigA.tile([128, 4, PLOC], F8, tag="bigA", name=f"xT_{sfx}")
            xsrc = xT_d.rearrange("(kt p) f -> p kt f", kt=4)
            # scalar queue: the early-critical blocks in consumption order;
            # gpsimd queue (behind the P1 weights): the late middle blocks
            for eng, blocks in [(nc.scalar, [(0, 2), (26, 28), (2, 8), (8, 14)]),
                                (nc.gpsimd, [(20, 26), (14, 20)])]:
                for lo, hi in blocks:
                    eng.dma_start(out=xT[:, :, lo * 224:hi * 224],
                                  in_=xsrc[:, :, lo * 224:hi * 224])
            Hv = bigB.tile([128, 4, PLOC], F8, tag="bigB", name=f"Hv_{sfx}")
            Hv5 = Hv.rearrange("p kt (h b w) -> p kt b h w", h=H, b=BL)

            with tc.tile_pool(name=f"g1{sfx}", bufs=1, space="PSUM") as gpool:
                cs = [state.tile([128, 2, 224], F16, tag=f"c1_{d}",
                                 name=f"c1_{d}_{sfx}") for d in range(2)]
                hprev = [None, None]
                for t in range(T):
                    pds, acts, hcurs = [], [], []
                    for d, L in enumerate(["vf", "vb"]):
                        pos = t if d == 0 else T - 1 - t
                        pd = gpool.tile([128, 4, 512], F32, tag=f"pd{d}",
                                        name=f"pd1_{d}_{t}_{sfx}")
                        _emit_matmuls(nc, pd, wih_sb[L], whh_sb[L],
                                      lambda q, _p=pos: xT[:, 2 * q:2 * q + 2,
                                                           _p * 224:(_p + 1) * 224],
                                      hprev[d], t)
                        pds.append((pd, pos))
                    for d in range(2):
                        name = f"1{d}_{t}_{sfx}"
                        IF, G, O = _emit_act(nc, scr, pds[d][0], t, name)
                        _emit_cell(nc, scr, IF, G, cs[d], t, name)
                        th = _emit_tail(nc, scr, cs[d], name)
                        hcur = scr.tile([128, 2, 224], F8, tag=f"ring{d}",
                                        bufs=2, name=f"h1_{d}_{t}_{sfx}")
                        nc.vector.tensor_mul(hcur, O, th)
                        pos = pds[d][1]
                        dst = Hv5[:, 2 * d:2 * d + 2, :, :, pos]
                        src = hcur.rearrange("p kt (b h) -> p kt b h", b=BL)
                        nc.gpsimd.tensor_copy(dst, src)
                        hprev[d] = hcur

            # --- P2: horizontal bi-LSTM (slab cols (h, b, w) both in and out) ---
            Hh = bigA.tile([128, 4, PLOC], F8, tag="bigA", name=f"Hh_{sfx}")
            with tc.tile_pool(name=f"g2{sfx}", bufs=1, space="PSUM") as gpool:
                cs = [state.tile([128, 2, 224], F16, tag=f"c2_{d}",
                                 name=f"c2_{d}_{sfx}") for d in range(2)]
                hprev = [None, None]
                for t in range(T):
                    pds, acts = [], []
                    for d, L in enumerate(["hf", "hb"]):
                        pos = t if d == 0 else T - 1 - t
                        pd = gpool.tile([128, 4, 512], F32, tag=f"pd{d}",
                                        name=f"pd2_{d}_{t}_{sfx}")
                        _emit_matmuls(nc, pd, wih_sb[L], whh_sb[L],
                                      lambda q, _p=pos: Hv[:, 2 * q:2 * q + 2,
                                                           _p * 224:(_p + 1) * 224],
                                      hprev[d], t)
                        pds.append((pd, pos))
                    for d in range(2):
                        name = f"2{d}_{t}_{sfx}"
                        IF, G, O = _emit_act(nc, scr, pds[d][0], t, name)
                        _emit_cell(nc, scr, IF, G, cs[d], t, name)
                        th = _emit_tail(nc, scr, cs[d], name)
                        pos = pds[d][1]
                        hslice = Hh[:, 2 * d:2 * d + 2, pos * 224:(pos + 1) * 224]
                        nc.vector.tensor_mul(hslice, O, th)
                        hprev[d] = hslice

            # --- P3: fc + softmax + transpose + einsum ---
            KT = bigB.tile([100, PLOC], BF16, tag="bigB", name=f"KT_{sfx}")
            with tc.tile_pool(name=f"p3{sfx}", bufs=2, space="PSUM") as pps:
                ci = 0
                for half in range(2):
                    for hr in range(H):
                        off = hr * 224 + half * 112
                        Lp = pps.tile([112, 100], F32, tag="L", name=f"L_{hr}_{half}_{sfx}")
                        for q in range(2):
                            nc.tensor.matmul(Lp,
                                             lhsT=Hh[:, 2 * q:2 * q + 2, off:off + 112],
                                             rhs=fcw_sb[:, 2 * q:2 * q + 2, :],
                                             start=(q == 0), stop=(q == 1),
                                             perf_mode=DR)
                        E = scr.tile([112, 100], F32, tag="E", bufs=3,
                                     name=f"E_{hr}_{half}_{sfx}")
                        Zs = scr.tile([112, 1], F32, tag="Z", bufs=3,
                                      name=f"Z_{hr}_{half}_{sfx}")
                        nc.scalar.activation(E, Lp, AF.Exp, accum_out=Zs)
                        rz = scr.tile([112, 1], F32, tag="rz", bufs=3,
                                      name=f"rz_{hr}_{half}_{sfx}")
                        nc.vector.reciprocal(rz, Zs)
                        Ka = scr.tile([112, 100], F32, tag="Ka", bufs=3,
                                      name=f"Ka_{hr}_{half}_{sfx}")
                        nc.vector.tensor_scalar_mul(Ka, E, rz)
                        KTp = pps.tile([100, 112], F32, tag="KTp",
                                       name=f"KTp_{hr}_{half}_{sfx}")
                        nc.tensor.transpose(KTp, Ka, ident)
                        # KT columns p = b*784 + hr*28 + w for these positions
                        dst = KT.rearrange("k (b hw) -> k b hw", b=BL)[
                            :, half * 4:(half + 1) * 4, hr * 28:(hr + 1) * 28]
                        if ci % 2 == 0:
                            nc.vector.tensor_copy(dst, KTp)
                        else:
                            nc.scalar.copy(dst, KTp)
                        ci += 1
                    # einsum for this half's samples (overlaps the other half's fc)
                    for b_i in range(half * 4, (half + 1) * 4):
                        for ct in range(4):
                            lhsT = patchT_sb[:, b_i, ct * 128:(ct + 1) * 128]
                            Op = pps.tile([128, 2, 512], F32, tag="O", bufs=2,
                                          name=f"O_{b_i}_{ct}_{sfx}")
                            for j2 in range(2):
                                nc.tensor.matmul(
                                    Op[:, j2, 0:392], lhsT=lhsT,
                                    rhs=KT[:, b_i * 784 + j2 * 392:
                                           b_i * 784 + (j2 + 1) * 392],
                                    start=True, stop=True)
                            ob = scr.tile([128, 2, 392], F32, tag="ob", bufs=3,
                                          name=f"ob_{b_i}_{ct}_{sfx}")
                            if ct % 2 == 0:
                                nc.vector.tensor_copy(ob, Op[:, :, 0:392])
                            else:
                                nc.scalar.copy(ob, Op[:, :, 0:392])
                            eng = nc.sync if ct % 2 == 0 else nc.scalar
                            eng.dma_start(
                                out=out_d[b_i, ct * 128:(ct + 1) * 128, :],
                                in_=ob)
            if debug and rep == reps - 1:
                nc.sync.dma_start(out=dbg_hv[:, :, :], in_=Hv)
                nc.sync.dma_start(out=dbg_hh[:, :, :], in_=Hh)
                nc.sync.dma_start(out=dbg_kt[:, :], in_=KT)

    nc.compile()
    return nc


_NC_CACHE = {}


def _get_nc(reps=1, debug=False, has_bias=False):
    key = (reps, debug, has_bias)
    if key not in _NC_CACHE:
        _NC_CACHE[key] = _build(reps=reps, debug=debug, has_bias=has_bias)
    return _NC_CACHE[key]


def _prep_core_inputs(x, weights_np):
    """Host-side marshalling for one core. x: [BL, C, H, W] f32."""
    f8 = ml_dtypes.float8_e4m3
    bf = ml_dtypes.bfloat16
    m = {}
    m["xT"] = np.ascontiguousarray(
        x.transpose(1, 3, 0, 2).reshape(C, PLOC)).astype(f8)
    m["patchT"] = np.ascontiguousarray(
        x[:, :, ::3, ::3].reshape(BL, C, 100).transpose(0, 2, 1)).astype(bf)
    m.update(weights_np)
    return m


def _prep_weights(inputs):
    f8 = ml_dtypes.float8_e4m3
    w = {}
    for L in _LSTMS:
        wih = np.asarray(inputs[L + "_Wih"], np.float32)
        whh = np.asarray(inputs[L + "_Whh"], np.float32)
        w[L + "_wih"] = np.ascontiguousarray(wih[_PERM].T).astype(f8)
        w[L + "_whh"] = np.ascontiguousarray(whh[_PERM].T).astype(f8)
    w["fcw"] = np.asarray(inputs["fc_W"], np.float32).astype(f8)
    return w


def run_cores(inputs, reps=1, debug=False):
    x = np.asarray(inputs["x"], np.float32)
    wnp = _prep_weights(inputs)
    nc = _get_nc(reps=reps, debug=debug)
    in_maps = [
        _prep_core_inputs(x[ci * BL:(ci + 1) * BL], wnp) for ci in range(N_CORES)
    ]
    res = run_bass_kernel_spmd(nc, in_maps, list(range(N_CORES)))
    return res


def kernel(**inputs) -> np.ndarray:
    res = run_cores(inputs)
    out = np.concatenate(
        [res.results[ci]["out"].reshape(BL, C, H, W) for ci in range(N_CORES)],
        axis=0)
    return out.astype(np.float32)


# revision 21
# speedup vs baseline: 1.1872x; 1.0196x over previous
"""PiCANet-G attention module as a Trainium2 Bass/Tile kernel.

Pure data-parallel over batch: 64 samples -> 8 cores x 8 samples.

Per core, three phases (all SBUF-resident):
  P1: vertical bi-LSTM over W (batch = 8*28 (b, h) rows, 28 steps, 2 dirs)
  P2: horizontal bi-LSTM over H (batch = 8*28 (b, w) rows)
  P3: fc -> softmax(100) -> per-sample einsum with the dilated 10x10 patch

All LSTM/fc matmuls run in fp8 (e4m3) with DoubleRow perf mode (2 K-tiles
per instruction, 0.5 cycles/row): half the PE time and half the matmul
instruction count vs bf16.  Gates accumulate in fp32 PSUM (one [128,4,512]
tile = 4 banks per direction, gate order i,f,o,g); nonlinearities run on
the Act engine as 3 instructions per (step, dir) (sigmoid over the i+f
banks fused, tanh(g), sigmoid(o)) plus tanh(c).  The element-wise cell
update runs on DVE in fp16 (2x mode).  Hidden state h is written in fp8:
P1 keeps h in a small per-direction ring (contiguous, feeds the next
step's recurrent matmul) while the Pool engine mirrors it into the big
Hv slab in (h, b, w) layout so P2's input matmuls read contiguous 3-D
slices; P2 writes its h directly into the Hh slab (its own recurrent
reads and P3's fc reads are both contiguous there).
"""

import numpy as np
import ml_dtypes
from contextlib import ExitStack

import concourse.bacc as bacc
import concourse.mybir as mybir
import concourse.tile as tile
from concourse.masks import make_identity
from concourse.bass_utils import run_bass_kernel_spmd

# problem shapes (hardcoded per contract)
B, C, H, W = 64, 512, 28, 28
HID = 256
N_CORES = 8
BL = B // N_CORES        # samples per core
NB = BL * H              # 224 rows per LSTM step
T = 28                   # steps per LSTM
PLOC = BL * H * W        # 6272 positions per core

BF16 = mybir.dt.bfloat16
F32 = mybir.dt.float32
F16 = mybir.dt.float16
F8 = mybir.dt.float8e4
AF = mybir.ActivationFunctionType
DR = mybir.MatmulPerfMode.DoubleRow

# torch gate order [i f g o] -> device order [i f o g] (sigmoids first)
_PERM = np.concatenate([np.arange(0, 512), np.arange(768, 1024), np.arange(512, 768)])

_LSTMS = ["vf", "vb", "hf", "hb"]


def _emit_matmuls(nc, pd, wih_sb, whh_sb, src_rhs, hprev, t):
    """PE work for one (step, dir): per gate-half region, a contiguous
    accumulation group of 2 fp8 DoubleRow ih matmuls (+1 hh when t>0)."""
    for g in range(4):
        for h in range(2):
            m = g * 2 + h
            out_ap = pd[:, g, h * 256: h * 256 + 224]
            for q in range(2):
                nc.tensor.matmul(
                    out_ap,
                    lhsT=wih_sb[:, 2 * q:2 * q + 2, m * 128:(m + 1) * 128],
                    rhs=src_rhs(q),
                    start=(q == 0), stop=(t == 0 and q == 1),
                    perf_mode=DR)
            if t > 0:
                nc.tensor.matmul(
                    out_ap,
                    lhsT=whh_sb[:, 0:2, m * 128:(m + 1) * 128],
                    rhs=hprev,
                    start=False, stop=True, perf_mode=DR)


def _emit_act(nc, scr, pd, t, name):
    """Act engine: sigmoid(i,f) fused, tanh(g), sigmoid(o). Returns tiles."""
    pdv = pd.rearrange("p g (h x) -> p g h x", h=2)
    IF = scr.tile([128, 2, 2, 224], F16, tag="IF", bufs=3, name=f"IF_{name}")
    nc.scalar.activation(IF, pdv[:, 0:2, :, 0:224], AF.Sigmoid)
    G = scr.tile([128, 2, 224], F16, tag="G", bufs=3, name=f"G_{name}")
    nc.scalar.activation(G, pdv[:, 3, :, 0:224], AF.Tanh)
    O = scr.tile([128, 2, 224], F16, tag="O", bufs=3, name=f"O_{name}")
    nc.scalar.activation(O, pdv[:, 2, :, 0:224], AF.Sigmoid)
    return IF, G, O


def _emit_cell(nc, scr, IF, G, c, t, name):
    """DVE cell update in fp16."""
    if t == 0:
        nc.vector.tensor_mul(c, IF[:, 0], G)
    else:
        nc.vector.tensor_mul(c, IF[:, 1], c)
        t1 = scr.tile([128, 2, 224], F16, tag="t1", bufs=3, name=f"t1_{name}")
        nc.vector.tensor_mul(t1, IF[:, 0], G)
        nc.vector.tensor_add(c, c, t1)


def _emit_tail(nc, scr, c, name):
    """Act: tanh(c); returns th for the h product."""
    th = scr.tile([128, 2, 224], F16, tag="th", bufs=3, name=f"th_{name}")
    nc.scalar.activation(th, c, AF.Tanh)
    return th


def _build(reps=1, debug=False, has_bias=False):
    nc = bacc.Bacc(None, target_bir_lowering=False)

    xT_d = nc.dram_tensor("xT", [C, PLOC], F8, kind="ExternalInput")
    w_d = {}
    for L in _LSTMS:
        w_d[L + "_wih"] = nc.dram_tensor(L + "_wih", [512, 1024], F8, kind="ExternalInput")
        w_d[L + "_whh"] = nc.dram_tensor(L + "_whh", [256, 1024], F8, kind="ExternalInput")
        if has_bias:
            w_d[L + "_bias"] = nc.dram_tensor(L + "_bias", [128, 8], F32, kind="ExternalInput")
    fcw_d = nc.dram_tensor("fcw", [512, 100], F8, kind="ExternalInput")
    patchT_d = nc.dram_tensor("patchT", [BL, 100, 512], BF16, kind="ExternalInput")
    out_d = nc.dram_tensor("out", [BL, C, H * W], BF16, kind="ExternalOutput")
    if debug:
        dbg_hv = nc.dram_tensor("dbg_hv", [128, 4, PLOC], F8, kind="ExternalOutput")
        dbg_hh = nc.dram_tensor("dbg_hh", [128, 4, PLOC], F8, kind="ExternalOutput")
        dbg_kt = nc.dram_tensor("dbg_kt", [100, PLOC], BF16, kind="ExternalOutput")

    with tile.TileContext(nc) as tc, ExitStack() as ctx:
        wpool = ctx.enter_context(tc.tile_pool(name="wpool", bufs=1))
        bigA = ctx.enter_context(tc.tile_pool(name="bigA", bufs=1))
        bigB = ctx.enter_context(tc.tile_pool(name="bigB", bufs=1))
        state = ctx.enter_context(tc.tile_pool(name="state", bufs=1))
        scr = ctx.enter_context(tc.tile_pool(name="scr", bufs=3))

        # --- load weights; both stage-1 dirs first (step 0 needs them) ---
        wih_sb, whh_sb = {}, {}
        for L in _LSTMS:
            wih_sb[L] = wpool.tile([128, 4, 1024], F8, name=f"wih_{L}")
            whh_sb[L] = wpool.tile([128, 2, 1024], F8, name=f"whh_{L}")
        # split the P1-critical loads across the sync/gpsimd queues (scalar
        # carries the xT stream); stage-2 weights and fc/patch data trail on
        # the sync queue (only needed at P2/P3)
        vb_src = w_d["vb_wih"].rearrange("(kt p) m -> p kt m", kt=4)
        nc.sync.dma_start(out=wih_sb["vf"],
                          in_=w_d["vf_wih"].rearrange("(kt p) m -> p kt m", kt=4))
        nc.sync.dma_start(out=wih_sb["vb"][:, 0:2], in_=vb_src[:, 0:2])
        for L in ["vf", "vb"]:
            nc.gpsimd.dma_start(out=whh_sb[L],
                                in_=w_d[L + "_whh"].rearrange("(kt p) m -> p kt m", kt=2))
        nc.gpsimd.dma_start(out=wih_sb["vb"][:, 2:4], in_=vb_src[:, 2:4])
        for L in ["hf", "hb"]:
            nc.sync.dma_start(out=wih_sb[L],
                              in_=w_d[L + "_wih"].rearrange("(kt p) m -> p kt m", kt=4))
            nc.sync.dma_start(out=whh_sb[L],
                              in_=w_d[L + "_whh"].rearrange("(kt p) m -> p kt m", kt=2))
        fcw_sb = wpool.tile([128, 4, 100], F8, name="fcw_sb")
        nc.sync.dma_start(out=fcw_sb, in_=fcw_d.rearrange("(kt p) n -> p kt n", kt=4))
        patchT_sb = wpool.tile([100, BL, 512], BF16, name="patchT_sb")
        nc.sync.dma_start(out=patchT_sb, in_=patchT_d.rearrange("b k c -> k b c"))
        ident = wpool.tile([112, 112], F32, name="ident")
        make_identity(nc, ident)
        # warm the Act LUTs during the DMA ramp so the first real activation
        # doesn't pay the table-load latency
        warm = wpool.tile([128, 2], F32, name="warm")
        nc.vector.memset(warm, 0.0)
        nc.scalar.activation(warm[:, 0:1], warm[:, 0:1], AF.Sigmoid)
        nc.scalar.activation(warm[:, 1:2], warm[:, 1:2], AF.Tanh)

        for rep in range(reps):
            sfx = f"r{rep}"
            # --- P1: vertical bi-LSTM (input cols (w, b, h); out slab (h, b, w)) ---
            xT = bigA.tile([128, 4, PLOC], F8, tag="bigA", name=f"xT_{sfx}")
            xsrc = xT_d.rearrange("(kt p) f -> p kt f", kt=4)
            # scalar queue: the early-critical blocks in consumption order;
            # gpsimd queue (behind the P1 weights): the late middle blocks
            for eng, blocks in [(nc.scalar, [(0, 2), (26, 28), (2, 8), (8, 14)]),
                                (nc.gpsimd, [(20, 26), (14, 20)])]:
                for lo, hi in blocks:
                    eng.dma_start(out=xT[:, :, lo * 224:hi * 224],
                                  in_=xsrc[:, :, lo * 224:hi * 224])
            Hv = bigB.tile([128, 4, PLOC], F8, tag="bigB", name=f"Hv_{sfx}")
            Hv5 = Hv.rearrange("p kt (h b w) -> p kt b h w", h=H, b=BL)

            with tc.tile_pool(name=f"g1{sfx}", bufs=1, space="PSUM") as gpool:
                cs = [state.tile([128, 2, 224], F16, tag=f"c1_{d}",
                                 name=f"c1_{d}_{sfx}") for d in range(2)]
                hprev = [None, None]
                for t in range(T):
                    pds, acts, hcurs = [], [], []
                    for d, L in enumerate(["vf", "vb"]):
                        pos = t if d == 0 else T - 1 - t
                        pd = gpool.tile([128, 4, 512], F32, tag=f"pd{d}",
                                        name=f"pd1_{d}_{t}_{sfx}")
                        _emit_matmuls(nc, pd, wih_sb[L], whh_sb[L],
                                      lambda q, _p=pos: xT[:, 2 * q:2 * q + 2,
                                                           _p * 224:(_p + 1) * 224],
                                      hprev[d], t)
                        pds.append((pd, pos))
                    for d in range(2):
                        name = f"1{d}_{t}_{sfx}"
                        IF, G, O = _emit_act(nc, scr, pds[d][0], t, name)
                        _emit_cell(nc, scr, IF, G, cs[d], t, name)
                        th = _emit_tail(nc, scr, cs[d], name)
                        hcur = scr.tile([128, 2, 224], F8, tag=f"ring{d}",
                                        bufs=2, name=f"h1_{d}_{t}_{sfx}")
                        nc.vector.tensor_mul(hcur, O, th)
                        pos = pds[d][1]
                        dst = Hv5[:, 2 * d:2 * d + 2, :, :, pos]
                        src = hcur.rearrange("p kt (b h) -> p kt b h", b=BL)
                        nc.gpsimd.tensor_copy(dst, src)
                        hprev[d] = hcur

            # --- P2: horizontal bi-LSTM (slab cols (h, b, w) both in and out) ---
            Hh = bigA.tile([128, 4, PLOC], F8, tag="bigA", name=f"Hh_{sfx}")
            with tc.tile_pool(name=f"g2{sfx}", bufs=1, space="PSUM") as gpool:
                cs = [state.tile([128, 2, 224], F16, tag=f"c2_{d}",
                                 name=f"c2_{d}_{sfx}") for d in range(2)]
                hprev = [None, None]
                for t in range(T):
                    pds, acts = [], []
                    for d, L in enumerate(["hf", "hb"]):
                        pos = t if d == 0 else T - 1 - t
                        pd = gpool.tile([128, 4, 512], F32, tag=f"pd{d}",
                                        name=f"pd2_{d}_{t}_{sfx}")
                        _emit_matmuls(nc, pd, wih_sb[L], whh_sb[L],
                                      lambda q, _p=pos: Hv[:, 2 * q:2 * q + 2,
                                                           _p * 224:(_p + 1) * 224],
                                      hprev[d], t)
                        pds.append((pd, pos))
                    for d in range(2):
                        name = f"2{d}_{t}_{sfx}"
                        IF, G, O = _emit_act(nc, scr, pds[d][0], t, name)
                        _emit_cell(nc, scr, IF, G, cs[d], t, name)
                        th = _emit_tail(nc, scr, cs[d], name)
                        pos = pds[d][1]
                        hslice = Hh[:, 2 * d:2 * d + 2, pos * 224:(pos + 1) * 224]
                        nc.vector.tensor_mul(hslice, O, th)
                        hprev[d] = hslice

            # --- P3: fc + softmax + transpose + einsum ---
            KT = bigB.tile([100, PLOC], BF16, tag="bigB", name=f"KT_{sfx}")
            with tc.tile_pool(name=f"p3{sfx}", bufs=2, space="PSUM") as pps:
                ci = 0
                for half in range(2):
                    for hr in range(H):
                        off = hr * 224 + half * 112
                        Lp = pps.tile([112, 100], F32, tag="L", name=f"L_{hr}_{half}_{sfx}")
                        for q in range(2):
                            nc.tensor.matmul(Lp,
                                             lhsT=Hh[:, 2 * q:2 * q + 2, off:off + 112],
                                             rhs=fcw_sb[:, 2 * q:2 * q + 2, :],
                                             start=(q == 0), stop=(q == 1),
                                             perf_mode=DR)
                        E = scr.tile([112, 100], F32, tag="E", bufs=3,
                                     name=f"E_{hr}_{half}_{sfx}")
                        Zs = scr.tile([112, 1], F32, tag="Z", bufs=3,
                                      name=f"Z_{hr}_{half}_{sfx}")
                        nc.scalar.activation(E, Lp, AF.Exp, accum_out=Zs)
                        rz = scr.tile([112, 1], F32, tag="rz", bufs=3,
                                      name=f"rz_{hr}_{half}_{sfx}")
                        nc.vector.reciprocal(rz, Zs)
                        Ka = scr.tile([112, 100], F32, tag="Ka", bufs=3,
                                      name=f"Ka_{hr}_{half}_{sfx}")
                        nc.vector.tensor_scalar_mul(Ka, E, rz)
                        KTp = pps.tile([100, 112], F32, tag="KTp",
                                       name=f"KTp_{hr}_{half}_{sfx}")
                        nc.tensor.transpose(KTp, Ka, ident)
                        # KT columns p = b*784 + hr*28 + w for these positions
                        dst = KT.rearrange("k (b hw) -> k b hw", b=BL)[
                            :, half * 4:(half + 1) * 4, hr * 28:(hr + 1) * 28]
                        if ci % 2 == 0:
                            nc.vector.tensor_copy(dst, KTp)
                        else:
                            nc.scalar.copy(dst, KTp)
                        ci += 1
                    # einsum for this half's samples (overlaps the other half's fc)
                    for b_i in range(half * 4, (half + 1) * 4):
                        for ct in range(4):
                            lhsT = patchT_sb[:, b_i, ct * 128:(ct + 1) * 128]
                            ob = scr.tile([128, 2, 392], BF16, tag="ob", bufs=3,
                                          name=f"ob_{b_i}_{ct}_{sfx}")
                            for j2 in range(2):
                                Op = pps.tile([128, 512], F32, tag="O", bufs=4,
                                              name=f"O_{b_i}_{ct}_{j2}_{sfx}")
                                nc.tensor.matmul(
                                    Op[:, 0:392], lhsT=lhsT,
                                    rhs=KT[:, b_i * 784 + j2 * 392:
                                           b_i * 784 + (j2 + 1) * 392],
                                    start=True, stop=True)
                                if (ct * 2 + j2) % 2 == 0:
                                    nc.vector.tensor_copy(ob[:, j2], Op[:, 0:392])
                                else:
                                    nc.scalar.copy(ob[:, j2], Op[:, 0:392])
                            eng = nc.sync if ct % 2 == 0 else nc.scalar
                            eng.dma_start(
                                out=out_d[b_i, ct * 128:(ct + 1) * 128, :],
                                in_=ob)
            if debug and rep == reps - 1:
                nc.sync.dma_start(out=dbg_hv[:, :, :], in_=Hv)
                nc.sync.dma_start(out=dbg_hh[:, :, :], in_=Hh)
                nc.sync.dma_start(out=dbg_kt[:, :], in_=KT)

    nc.compile()
    return nc


_NC_CACHE = {}


def _get_nc(reps=1, debug=False, has_bias=False):
    key = (reps, debug, has_bias)
    if key not in _NC_CACHE:
        _NC_CACHE[key] = _build(reps=reps, debug=debug, has_bias=has_bias)
    return _NC_CACHE[key]


def _prep_core_inputs(x, weights_np):
    """Host-side marshalling for one core. x: [BL, C, H, W] f32."""
    f8 = ml_dtypes.float8_e4m3
    bf = ml_dtypes.bfloat16
    m = {}
    m["xT"] = np.ascontiguousarray(
        x.transpose(1, 3, 0, 2).reshape(C, PLOC)).astype(f8)
    m["patchT"] = np.ascontiguousarray(
        x[:, :, ::3, ::3].reshape(BL, C, 100).transpose(0, 2, 1)).astype(bf)
    m.update(weights_np)
    return m


def _prep_weights(inputs):
    f8 = ml_dtypes.float8_e4m3
    w = {}
    for L in _LSTMS:
        wih = np.asarray(inputs[L + "_Wih"], np.float32)
        whh = np.asarray(inputs[L + "_Whh"], np.float32)
        w[L + "_wih"] = np.ascontiguousarray(wih[_PERM].T).astype(f8)
        w[L + "_whh"] = np.ascontiguousarray(whh[_PERM].T).astype(f8)
    w["fcw"] = np.asarray(inputs["fc_W"], np.float32).astype(f8)
    return w


def run_cores(inputs, reps=1, debug=False):
    x = np.asarray(inputs["x"], np.float32)
    wnp = _prep_weights(inputs)
    nc = _get_nc(reps=reps, debug=debug)
    in_maps = [
        _prep_core_inputs(x[ci * BL:(ci + 1) * BL], wnp) for ci in range(N_CORES)
    ]
    res = run_bass_kernel_spmd(nc, in_maps, list(range(N_CORES)))
    return res


def kernel(**inputs) -> np.ndarray:
    res = run_cores(inputs)
    out = np.concatenate(
        [res.results[ci]["out"].reshape(BL, C, H, W) for ci in range(N_CORES)],
        axis=0)
    return out.astype(np.float32)
